# revision 44
# baseline (speedup 1.0000x reference)
"""Trainium2 Bass kernel for nn_Encoder_37340445671714 (video ViT encoder).

Sharding: 8 cores = 4 batch elements x 2 sequence halves (788 tokens each),
with a per-core LOCAL token order of [own 788 | partner 788] so the program
is identical on every core (SPMD).

Per layer:
  - pair exchange via ReduceScatter(add) of bf16 x (input duplicated): both
    cores receive the pair SUM; partner half = sum - own, written fp8 into
    x_all[:, :, OWN:]. The collective hides under own-half attention work.
  - Q/K/V projections and AV run in fp8e4 DoubleRow (2 contraction k-tiles
    per instruction); scores, Wo and the FFN stay bf16 (error budget).
  - flash attention, own-first k-tile order; one Exp per k-tile covering
    both heads of the pair (scores for both heads land in one 2-plane psum
    tile). V weights host-extended per head with a zero-weight/bias-sv
    block so AV also produces sv*Z (sv cancels in o = sv*num / (sv*Z));
    1/Z via DVE divide after an anti-diagonal f32r swap matmul. The ones
    columns of V' are memset once; per-tile V writes touch only V columns.
  - projection biases for Q/K-own applied on ScalarE (per-partition bias),
    FFN relu+bias on ScalarE; LayerNorm stats via ones-matmul partition
    sums (sumsq in bf16); LN2 also emits the fp8 x copy for the next layer.
Weight/scale prep happens on the host (free). Output transposed on the PE.
"""

import math

import numpy as np
import ml_dtypes

import concourse.bass as bass
import concourse.tile as tile
from concourse import mybir
from concourse.bass_utils import run_bass_kernel_spmd

F32 = mybir.dt.float32
F32R = mybir.dt.float32r
BF16 = mybir.dt.bfloat16
F8 = mybir.dt.float8e4
AF = mybir.ActivationFunctionType
OP = mybir.AluOpType
DR = mybir.MatmulPerfMode.DoubleRow

# problem dims
B, L, C, H, W = 4, 8, 3, 224, 224
PH = PW = 16
D = 512
NH = 8
DK = 64
FF = 2048
NL = 6
NP = (H // PH) * (W // PW)  # 196
S = L * (NP + 1)  # 1576
PD = PH * PW * C  # 768
OWN = S // 2  # 788 tokens per core
LN_EPS = 1e-5
SV = 32.0  # fp8 scale for extended V (cancels in softmax normalize)

DC = D // 128  # 4
PDC = PD // 128  # 6
FTC = FF // 128  # 16

KT = [(i * 128, 128) for i in range(S // 128)] + [(S - S % 128, S % 128)]
PAIRS = [(0, 1), (2, 3), (4, 5), (6, 7), (8, 9), (10, 11)]
SINGLE = 12
HQ = OWN // 2  # 394 (half of the own-token range; psum-bank-sized chunks)

N_CORES = 8
REPLICA_GROUPS = [[0, 1], [2, 3], [4, 5], [6, 7]]


def legalize_waits(nc):
    """Split multi-wait instructions into preceding single-wait NoOps."""
    n_split = 0
    for f in nc.m.functions:
        for bb in f.blocks:
            insts = list(bb.instructions)
            new_insts = []
            changed = False
            for inst in insts:
                si = inst.sync_info
                if si is not None and len(si.on_wait) > 1:
                    waits = list(si.on_wait)
                    for w in waits[:-1]:
                        nop = mybir.InstNoOp(
                            name=nc.get_next_instruction_name(),
                            engine=inst.engine,
                            ins=[],
                            outs=[],
                        )
                        nop.sync_info = mybir.SyncInfo(on_wait=[w], on_update=[])
                        new_insts.append(nop)
                        n_split += 1
                    inst.sync_info = mybir.SyncInfo(
                        on_wait=[waits[-1]], on_update=list(si.on_update)
                    )
                    changed = True
                new_insts.append(inst)
            if changed:
                bb.instructions = new_insts
    return n_split


def _bcast_ap(ap_1d, parts=128):
    return bass.AP(
        tensor=ap_1d.tensor, offset=ap_1d.offset, ap=[[0, parts]] + list(ap_1d.ap)
    )


def _vcols(base, ksz=None):
    """Two APs (even-head, odd-head) selecting the V columns (per 256
    lanes: [0,64) then [192,256)) of an AP whose last dim is [1, 1024]."""
    ap = [list(d) for d in base.ap]
    assert ap[-1][0] == 1 and ap[-1][1] == NH * 128
    p = ap[0]
    if ksz is not None:
        p = [p[0], ksz]
    return [
        bass.AP(tensor=base.tensor, offset=base.offset + off,
                ap=[p, [256, 4], [1, 64]])
        for off in (0, 192)
    ]


def _vcols_dr(v_f8, pi, j, ksz):
    """Two destination APs inside v_dr [128, pair, head, parity, 128]:
    even head h -> cols [0,64) of block h*256+j*128, odd -> [64,128)."""
    base = v_f8[:]
    pstr = base.ap[0][0]
    off = base.offset + pi * 2048 + j * 128
    return [
        bass.AP(tensor=base.tensor, offset=off + o2,
                ap=[[pstr, ksz], [512, 4], [1, 64]])
        for o2 in (0, 320)
    ]


def build_kernel(exp_scales, s_v):
    nc = bass.Bass(
        "TRN2", target_bir_lowering=False, debug=False, num_devices=N_CORES
    )

    pat = nc.dram_tensor("pat", [PD, OWN], BF16, kind="ExternalInput").ap()
    addv = nc.dram_tensor("addv", [D, OWN], F32, kind="ExternalInput").ap()
    wembT = nc.dram_tensor("wembT", [PD, D], BF16, kind="ExternalInput").ap()
    wqT = nc.dram_tensor("wqT8", [NL, 128, 2 * DC * 2 * 128], F8,
                         kind="ExternalInput").ap()
    wkT = nc.dram_tensor("wkT8", [NL, 128, 2 * DC * 2 * 128], F8,
                         kind="ExternalInput").ap()
    wvT = nc.dram_tensor("wvxT8", [NL, D, NH * 128], F8, kind="ExternalInput").ap()
    woT = nc.dram_tensor("woT", [NL, D, D], BF16, kind="ExternalInput").ap()
    w1T = nc.dram_tensor("w1T", [NL, D, FF], BF16, kind="ExternalInput").ap()
    w2T = nc.dram_tensor("w2T", [NL, FF, D], BF16, kind="ExternalInput").ap()
    bq = nc.dram_tensor("bq", [NL, D], F32, kind="ExternalInput").ap()
    bk = nc.dram_tensor("bk", [NL, D], F32, kind="ExternalInput").ap()
    bv = nc.dram_tensor("bvx", [NL, NH * 128], F32, kind="ExternalInput").ap()
    bo = nc.dram_tensor("bo", [NL, D], F32, kind="ExternalInput").ap()
    b1 = nc.dram_tensor("b1", [NL, FF], F32, kind="ExternalInput").ap()
    b2 = nc.dram_tensor("b2", [NL, D], F32, kind="ExternalInput").ap()
    g1 = nc.dram_tensor("g1", [NL, D], F32, kind="ExternalInput").ap()
    be1 = nc.dram_tensor("be1", [NL, D], F32, kind="ExternalInput").ap()
    g2 = nc.dram_tensor("g2", [NL, D], F32, kind="ExternalInput").ap()
    be2 = nc.dram_tensor("be2", [NL, D], F32, kind="ExternalInput").ap()
    ident = nc.dram_tensor("ident", [128, 128], BF16, kind="ExternalInput").ap()
    swapid = nc.dram_tensor("swapid", [128, 128], F32R, kind="ExternalInput").ap()
    xout = nc.dram_tensor("xout", [OWN, D], F32, kind="ExternalOutput").ap()

    with tile.TileContext(nc) as tc:
        with (
            tc.tile_pool(name="const", bufs=1) as constp,
            tc.tile_pool(name="wsmall", bufs=1) as wsmall,
            tc.tile_pool(name="wff", bufs=2) as wff,
            tc.tile_pool(name="xp", bufs=2) as xp,
            tc.tile_pool(name="xa", bufs=2) as xap,
            tc.tile_pool(name="kv", bufs=1) as kvp,
            tc.tile_pool(name="vp", bufs=1) as vp,
            tc.tile_pool(name="qo", bufs=1) as qop,
            tc.tile_pool(name="zp", bufs=1) as zp,
            tc.tile_pool(name="zbp", bufs=1) as zbp,
            tc.tile_pool(name="xs", bufs=1) as xsp,
            tc.tile_pool(name="big", bufs=1) as bigp,
            tc.tile_pool(name="exps", bufs=6) as expp,
            tc.tile_pool(name="stat", bufs=5) as statp,
            tc.tile_pool(name="rz", bufs=3) as rzp,
            tc.tile_pool(name="bias", bufs=2) as biasp,
            tc.tile_pool(name="psA", bufs=1, space="PSUM") as psA,
            tc.tile_pool(name="psB", bufs=2, space="PSUM") as psB,
            tc.tile_pool(name="dram", bufs=2, space="DRAM") as dramp,
        ):
            P = dict(
                constp=constp, wsmall=wsmall, wff=wff, xp=xp, xap=xap, kvp=kvp,
                vp=vp, qop=qop, zp=zp, zbp=zbp, xsp=xsp, bigp=bigp, expp=expp,
                statp=statp, rzp=rzp, biasp=biasp, psA=psA, psB=psB, dramp=dramp,
            )
            dram_in = dict(
                pat=pat, addv=addv, wembT=wembT, wqT=wqT, wkT=wkT, wvT=wvT,
                woT=woT, w1T=w1T, w2T=w2T, bq=bq, bk=bk, bv=bv, bo=bo, b1=b1,
                swapid=swapid, b2=b2, g1=g1, be1=be1, g2=g2, be2=be2,
                ident=ident, xout=xout,
            )
            ones_bf = constp.tile([128, 128], BF16, name="ones_bf")
            nc.vector.memset(ones_bf[:], 1.0)
            ones_r = constp.tile([128, 128], F32R, name="ones_r")
            nc.vector.tensor_scalar_add(ones_r[:], ones_bf[:], 0.0)
            ident_sb = constp.tile([128, 128], BF16, name="ident_sb")
            nc.sync.dma_start(ident_sb[:], ident[:])
            eps_sb = constp.tile([128, 1], F32, name="eps_sb")
            nc.vector.memset(eps_sb[:], LN_EPS)
            swap_sb = constp.tile([128, 128], F32R, name="swap_sb")
            nc.sync.dma_start(swap_sb[:], swapid[:])
            onesw = constp.tile([128, OWN], F32, name="onesw")
            nc.vector.memset(onesw[:], 1.0)
            P["ones_bf"] = ones_bf
            P["ones_r"] = ones_r
            P["ident_sb"] = ident_sb
            P["eps_sb"] = eps_sb
            P["swap_sb"] = swap_sb
            P["onesw"] = onesw

            # extended-V tiles live for the whole kernel; ones columns are
            # constant (= s_v) and written exactly once here.
            v_f8 = vp.tile([128, len(PAIRS), NH, 2, 128], F8, name="v_f8")
            v_12 = vp.tile([128, NH * 128], F8, name="v_12")
            P["v_f8"] = v_f8
            P["v_12"] = v_12
            vb = v_f8[:]
            pstr = vb.ap[0][0]
            # ones columns: even heads at h*256+64, odd heads at h*256 (+j*128)
            for base_off in (64, 256):
                nc.vector.memset(
                    bass.AP(
                        tensor=vb.tensor, offset=vb.offset + base_off,
                        ap=[[pstr, 128], [2048, len(PAIRS)], [512, 4],
                            [128, 2], [1, 64]],
                    ),
                    s_v,
                )
            v12b = v_12[:]
            nc.vector.memset(
                bass.AP(
                    tensor=v12b.tensor, offset=v12b.offset + 64,
                    ap=[[v12b.ap[0][0], 128], [256, 4], [1, 128]],
                ),
                s_v,
            )

            x_bf, x_all = _embed(nc, P, dram_in)
            for l in range(NL):
                x_bf, x_all = _one_layer(
                    nc, P, dram_in, l, x_bf, x_all, exp_scales[l]
                )
            _tail(nc, P, dram_in, x_bf)
    return nc


def _embed(nc, P, dr):
    bigp, zp, wff, xp, psB = P["bigp"], P["zp"], P["wff"], P["xp"], P["psB"]
    pat_sb = bigp.tile([128, FTC, OWN], BF16, tag="h", name="pat_sb")
    nc.sync.dma_start(
        pat_sb[:, :PDC, :], dr["pat"].rearrange("(ko p) t -> p ko t", p=128)
    )
    addv_sb = zp.tile([128, DC, OWN], F32, tag="z", name="addv_sb")
    nc.sync.dma_start(addv_sb[:], dr["addv"].rearrange("(co p) t -> p co t", p=128))
    wemb_sb = wff.tile([128, PDC, D], BF16, tag="wff", name="wemb_sb")
    nc.sync.dma_start(wemb_sb[:], dr["wembT"].rearrange("(ko p) d -> p ko d", p=128))

    x_bf = xp.tile([128, DC, OWN], BF16, tag="x", name="x_emb")
    x_all = P["xap"].tile([128, DC, S], F8, tag="xa", name="x_all_0")
    for dt in range(DC):
        ps = psB.tile([128, 2, 512], F32, tag="s", name="ps_emb")
        for ci in range(2):
            for kt in range(PDC):
                nc.tensor.matmul(
                    ps[:, ci, :HQ],
                    wemb_sb[:, kt, dt * 128 : (dt + 1) * 128],
                    pat_sb[:, kt, ci * HQ : (ci + 1) * HQ],
                    start=(kt == 0),
                    stop=(kt == PDC - 1),
                )
        nc.vector.tensor_tensor(
            x_bf[:, dt, :].rearrange("p (a b) -> p a b", a=2),
            ps[:, :, :HQ],
            addv_sb[:, dt, :].rearrange("p (a b) -> p a b", a=2),
            OP.add,
        )
        nc.vector.tensor_tensor(
            x_all[:, dt, 0:OWN].rearrange("p (a b) -> p a b", a=2),
            ps[:, :, :HQ],
            addv_sb[:, dt, :].rearrange("p (a b) -> p a b", a=2),
            OP.add,
        )
    return x_bf, x_all


def _load_layer_params(nc, P, dr, l):
    biasp, wsmall, wff = P["biasp"], P["wsmall"], P["wff"]
    prm = {}
    for nm in ["bq", "bk", "bo", "b2", "g1", "be1", "g2", "be2"]:
        t = biasp.tile([128, DC], F32, tag=nm, name=nm + "_sb")
        nc.sync.dma_start(t[:], dr[nm][l].rearrange("(o p) -> p o", p=128))
        prm[nm] = t
    b1_sb = biasp.tile([128, FTC], F32, tag="b1", name="b1_sb")
    nc.sync.dma_start(b1_sb[:], dr["b1"][l].rearrange("(o p) -> p o", p=128))
    prm["b1"] = b1_sb
    bv_bc = biasp.tile([128, NH * 128], BF16, tag="bvb", name="bv_bc")
    nc.gpsimd.dma_start(bv_bc[:], _bcast_ap(dr["bv"][l]))
    prm["bv_bc"] = bv_bc
    for nm, key in [("wq", "wqT"), ("wk", "wkT")]:
        t = wsmall.tile([128, 2, DC, 2, 128], F8, tag=nm, name=nm + "_sb")
        nc.sync.dma_start(
            t[:].rearrange("p a b c d -> p (a b c d)"), dr[key][l]
        )
        prm[nm] = t
    wo = wsmall.tile([128, DC, D], BF16, tag="wo", name="wo_sb")
    nc.sync.dma_start(wo[:], dr["woT"][l].rearrange("(co p) d -> p co d", p=128))
    prm["wo"] = wo
    wv = wsmall.tile([128, DC, NH * 128], F8, tag="wv", name="wv_sb")
    nc.sync.dma_start(wv[:], dr["wvT"][l].rearrange("(co p) d -> p co d", p=128))
    prm["wv"] = wv
    w1_sb = wff.tile([128, DC, FF], BF16, tag="wff", name="w1_sb")
    nc.sync.dma_start(w1_sb[:], dr["w1T"][l].rearrange("(co p) f -> p co f", p=128))
    prm["w1"] = w1_sb
    w2_sb = wff.tile([128, FTC, D], BF16, tag="wff", name="w2_sb")
    nc.sync.dma_start(w2_sb[:], dr["w2T"][l].rearrange("(fo p) d -> p fo d", p=128))
    prm["w2"] = w2_sb
    return prm


def _exchange_start(nc, P, x_bf):
    dramp = P["dramp"]
    xg_in = dramp.tile([2, DC, 128, OWN], BF16, tag="agi", name="xg_in")
    for s_ in range(2):
        nc.sync.dma_start(xg_in[s_].rearrange("c p t -> p c t"), x_bf[:])
    xg_sum = dramp.tile([DC, 128, OWN], BF16, tag="ago", name="xg_sum")
    nc.gpsimd.collective_compute(
        "ReduceScatter",
        OP.add,
        replica_groups=REPLICA_GROUPS,
        ins=[xg_in[:].opt()],
        outs=[xg_sum[:].opt()],
    )
    return xg_sum


def _exchange_finish(nc, P, xg_sum, x_bf, x_all):
    xsum_sb = P["xsp"].tile([128, DC, OWN], BF16, tag="xs", name="xsum_sb")
    nc.sync.dma_start(xsum_sb[:], xg_sum.rearrange("c p t -> p c t"))
    for h in range(2):
        cs = slice(2 * h, 2 * h + 2)
        nc.vector.tensor_tensor(
            x_all[:, cs, OWN:S], xsum_sb[:, cs, :], x_bf[:, cs, :], OP.subtract
        )


def _proj_dr(nc, P, w_f8, x_src, out_sb, bias_sb, t0, tn, act_bias):
    """out_sb[:, dt, t0:t0+tn] (bf16) = fp8 DoubleRow proj + bias."""
    psB = P["psB"]
    hn = tn // 2
    for dt in range(DC):
        ps = psB.tile([128, 2, 512], F32, tag="s", name="ps_p")
        for ci in range(2):
            q0 = t0 + ci * hn
            for c2 in range(2):
                nc.tensor.matmul(
                    ps[:, ci, :hn],
                    w_f8[:, c2, dt, :, :],
                    x_src[:, 2 * c2 : 2 * c2 + 2, q0 : q0 + hn],
                    start=(c2 == 0),
                    stop=(c2 == 1),
                    perf_mode=DR,
                )
        dst = out_sb[:, dt, t0 : t0 + tn].rearrange("p (a b) -> p a b", a=2)
        if act_bias:
            nc.scalar.activation(
                dst, ps[:, :, :hn], AF.Identity, bias=bias_sb[:, dt : dt + 1]
            )
        else:
            nc.vector.tensor_scalar_add(dst, ps[:, :, :hn], bias_sb[:, dt : dt + 1])


def _v_proj_tile(nc, P, prm, x_all, t, v_dst_cols, ps_pool_tag):
    """V columns of the extended-V tile for k-tile t (+ bias), fp8 out."""
    k0, ksz = KT[t]
    if ps_pool_tag == "s":
        pst = P["psB"].tile([128, 2, 512], F32, tag="s", name="ps_v")
    else:
        pst = P["psA"].tile([128, 2, 512], F32, tag=ps_pool_tag, name="ps_v")
    for half in range(2):
        for ct in range(DC):
            nc.tensor.matmul(
                pst[:ksz, half, :],
                x_all[:, ct, k0 : k0 + ksz],
                prm["wv"][:, ct, half * 512 : (half + 1) * 512],
                start=(ct == 0),
                stop=(ct == DC - 1),
            )
    ps_flat = bass.AP(
        tensor=pst[:].tensor, offset=pst[:].offset,
        ap=[list(pst[:].ap[0])] + [[1, 1024]],
    )
    for dst, src, bv in zip(
        v_dst_cols, _vcols(ps_flat, ksz), _vcols(prm["bv_bc"][:], ksz)
    ):
        nc.vector.tensor_tensor(dst, src, bv, OP.add)


def _flash_pairs(nc, P, prm, kT, qT, p1, p2, sc_exp, hpair, pis, first_pi):
    psB, expp = P["psB"], P["expp"]
    hdt = hpair
    v_f8 = P["v_f8"]
    for pi in pis:
        ta, tb = PAIRS[pi]
        for sub in range(2):
            hp = sub * 64
            head = 2 * hpair + sub
            e_t = expp.tile([128, 2, 2, HQ], F8, tag="e", name="e_t")
            for j, t in enumerate((ta, tb)):
                k0, ksz = KT[t]
                s_ps = psB.tile([128, 2, 512], F32, tag="s", name="s_ps")
                for ci in range(2):
                    nc.tensor.matmul(
                        s_ps[:ksz, ci, :HQ],
                        kT[hp : hp + 64, hdt, k0 : k0 + ksz],
                        qT[hp : hp + 64, hdt, ci * HQ : (ci + 1) * HQ],
                        start=True,
                        stop=True,
                    )
                nc.scalar.activation(
                    e_t[:ksz, :, j, :], s_ps[:ksz, :, 0:HQ], AF.Exp, scale=sc_exp
                )
            dst = p1 if sub == 0 else p2
            for ci in range(2):
                nc.tensor.matmul(
                    dst[:, ci, :HQ],
                    v_f8[:, pi, head, :, :],
                    e_t[:, ci, :, :],
                    start=(pi == first_pi),
                    stop=False,
                    perf_mode=DR,
                )


def _flash_single_and_rz(nc, P, prm, kT, qT, p1, p2, sc_exp, hpair, oT):
    psB, expp, rzp = P["psB"], P["expp"], P["rzp"]
    hdt = hpair
    k0, ksz = KT[SINGLE]
    for sub in range(2):
        hp = sub * 64
        head = 2 * hpair + sub
        e_t = expp.tile([128, 2, 2, HQ], F8, tag="e", name="e_t12")
        s_ps = psB.tile([128, 2, 512], F32, tag="s", name="s_p12")
        for ci in range(2):
            nc.tensor.matmul(
                s_ps[:ksz, ci, :HQ],
                kT[hp : hp + 64, hdt, k0 : k0 + ksz],
                qT[hp : hp + 64, hdt, ci * HQ : (ci + 1) * HQ],
                start=True,
                stop=True,
            )
        nc.scalar.activation(
            e_t[:ksz, :, 0, :], s_ps[:ksz, :, 0:HQ], AF.Exp, scale=sc_exp
        )
        dst = p1 if sub == 0 else p2
        for ci in range(2):
            nc.tensor.matmul(
                dst[:, ci, :HQ],
                P["v_12"][:ksz, head * 128 : (head + 1) * 128],
                e_t[:ksz, ci, 0, :],
                start=False,
                stop=True,
            )
    # 1/Z: Z_o in p2[0:64], Z_e in p1[64:128]; ln+exp base-aligned, then
    # swap halves with the f32r anti-diagonal-identity matmul.
    lnmix = rzp.tile([128, 2, HQ], F32, tag="rz", name="lnmix")
    nc.scalar.activation(lnmix[0:64, :, :], p2[0:64, :, 0:HQ], AF.Ln)
    nc.scalar.activation(lnmix[64:128, :, :], p1[64:128, :, 0:HQ], AF.Ln)
    rzmix = rzp.tile([128, 2, HQ], F32R, tag="rz", name="rzmix")
    nc.scalar.activation(rzmix[:], lnmix[:], AF.Exp, scale=-1.0)
    rsw = psB.tile([128, 2, 512], F32, tag="s", name="rsw")
    for ci in range(2):
        nc.tensor.matmul(
            rsw[:, ci, :HQ], P["swap_sb"][:], rzmix[:, ci, :], start=True,
            stop=True,
        )
    rzs = rzp.tile([128, 2, HQ], F32, tag="rz", name="rzs")
    nc.vector.tensor_scalar_add(rzs[:], rsw[:, :, 0:HQ], 0.0)
    nc.vector.tensor_tensor(
        oT[0:64, hdt, :].rearrange("p (a b) -> p a b", a=2),
        p1[0:64, :, 0:HQ], rzs[0:64, :, :], OP.mult,
    )
    nc.vector.tensor_tensor(
        oT[64:128, hdt, :].rearrange("p (a b) -> p a b", a=2),
        p2[64:128, :, 0:HQ], rzs[64:128, :, :], OP.mult,
    )


def _attention(nc, P, prm, x_all, sc_exp, xg_sum, x_bf):
    psA, qop = P["psA"], P["qop"]
    qT = qop.tile([128, DC, OWN], BF16, tag="qT", name="qT")
    _proj_dr(nc, P, prm["wq"], x_all, qT, prm["bq"], 0, OWN, act_bias=True)
    kT = P["kvp"].tile([128, DC, S], BF16, tag="kv", name="kT")
    _proj_dr(nc, P, prm["wk"], x_all, kT, prm["bk"], 0, OWN, act_bias=True)

    v_f8 = P["v_f8"]
    for pi in range(3):
        for j in range(2):
            t = PAIRS[pi][j]
            _v_proj_tile(
                nc, P, prm, x_all, t, _vcols_dr(v_f8, pi, j, KT[t][1]),
                "o" if (2 * pi + j) % 2 == 0 else "zz",
            )

    oT = qop.tile([128, DC, OWN], BF16, tag="oT", name="oT")
    # hpair 0: own pairs first (the exchange is still in flight)
    p1 = psA.tile([128, 2, 512], F32, tag="o", name="p1_0")
    p2 = psA.tile([128, 2, 512], F32, tag="zz", name="p2_0")
    _flash_pairs(nc, P, prm, kT, qT, p1, p2, sc_exp, 0, range(0, 3), 0)

    # partner half lands; finish exchange, late K/V
    _exchange_finish(nc, P, xg_sum, x_bf, x_all)
    _proj_dr(nc, P, prm["wk"], x_all, kT, prm["bk"], OWN, OWN, act_bias=False)
    for pi in range(3, len(PAIRS)):
        for j in range(2):
            t = PAIRS[pi][j]
            _v_proj_tile(
                nc, P, prm, x_all, t, _vcols_dr(v_f8, pi, j, KT[t][1]), "s"
            )
    _v_proj_tile(
        nc, P, prm, x_all, SINGLE, _vcols(P["v_12"][:], KT[SINGLE][1]), "s"
    )

    _flash_pairs(nc, P, prm, kT, qT, p1, p2, sc_exp, 0, range(3, len(PAIRS)), 0)
    _flash_single_and_rz(nc, P, prm, kT, qT, p1, p2, sc_exp, 0, oT)
    for hpair in range(1, NH // 2):
        p1 = psA.tile([128, 2, 512], F32, tag="o", name=f"p1_{hpair}")
        p2 = psA.tile([128, 2, 512], F32, tag="zz", name=f"p2_{hpair}")
        _flash_pairs(
            nc, P, prm, kT, qT, p1, p2, sc_exp, hpair, range(len(PAIRS)), 0
        )
        _flash_single_and_rz(nc, P, prm, kT, qT, p1, p2, sc_exp, hpair, oT)
    return oT


def _residual_proj(nc, P, w_sb, rhs_T, bias_sb, x_bf, z_out):
    """z_out (f32) = bf16 W^T proj of rhs_T + bias + x_bf (residual)."""
    psB = P["psB"]
    for dt in range(DC):
        ps = psB.tile([128, 2, 512], F32, tag="s", name="ps_r")
        for ci in range(2):
            for ct in range(DC):
                nc.tensor.matmul(
                    ps[:, ci, :HQ],
                    w_sb[:, ct, dt * 128 : (dt + 1) * 128],
                    rhs_T[:, ct, ci * HQ : (ci + 1) * HQ],
                    start=(ct == 0),
                    stop=(ct == DC - 1),
                )
        nc.vector.scalar_tensor_tensor(
            z_out[:, dt, :].rearrange("p (a b) -> p a b", a=2),
            ps[:, :, :HQ],
            bias_sb[:, dt : dt + 1],
            x_bf[:, dt, :].rearrange("p (a b) -> p a b", a=2),
            OP.add,
            OP.add,
        )


def _ffn(nc, P, prm, x_bf, z_out):
    psA, bigp = P["psA"], P["bigp"]
    h_bf = bigp.tile([128, FTC, OWN], BF16, tag="h", name="h_bf")
    for ft in range(FTC):
        ps = psA.tile(
            [128, 2, 512], F32, tag=("o" if ft % 2 == 0 else "zz"), name="ps_h"
        )
        for ci in range(2):
            for ct in range(DC):
                nc.tensor.matmul(
                    ps[:, ci, :HQ],
                    prm["w1"][:, ct, ft * 128 : (ft + 1) * 128],
                    x_bf[:, ct, ci * HQ : (ci + 1) * HQ],
                    start=(ct == 0),
                    stop=(ct == DC - 1),
                )
        nc.scalar.activation(
            h_bf[:, ft, :].rearrange("p (a b) -> p a b", a=2),
            ps[:, :, :HQ],
            AF.Relu,
            bias=prm["b1"][:, ft : ft + 1],
        )
    for dt in range(DC):
        ps2 = psA.tile(
            [128, 2, 512], F32, tag=("o" if dt % 2 == 0 else "zz"), name="ps_f"
        )
        for ci in range(2):
            for ft in range(FTC):
                nc.tensor.matmul(
                    ps2[:, ci, :HQ],
                    prm["w2"][:, ft, dt * 128 : (dt + 1) * 128],
                    h_bf[:, ft, ci * HQ : (ci + 1) * HQ],
                    start=(ft == 0),
                    stop=(ft == FTC - 1),
                )
        nc.vector.scalar_tensor_tensor(
            z_out[:, dt, :].rearrange("p (a b) -> p a b", a=2),
            ps2[:, :, :HQ],
            prm["b2"][:, dt : dt + 1],
            x_bf[:, dt, :].rearrange("p (a b) -> p a b", a=2),
            OP.add,
            OP.add,
        )


def _one_layer(nc, P, dr, l, x_bf, x_all, sc_exp):
    xg_sum = _exchange_start(nc, P, x_bf)
    prm = _load_layer_params(nc, P, dr, l)
    oT = _attention(nc, P, prm, x_all, sc_exp, xg_sum, x_bf)
    z = P["zp"].tile([128, DC, OWN], F32R, tag="z", name=f"z1_{l}")
    _residual_proj(nc, P, prm["wo"], oT, prm["bo"], x_bf, z)
    x_bf1 = P["xp"].tile([128, DC, OWN], BF16, tag="x", name=f"x_ln1_{l}")
    _layernorm(nc, P, z, x_bf1, None, prm["g1"], prm["be1"])
    z = P["zp"].tile([128, DC, OWN], F32R, tag="z", name=f"z2_{l}")
    _ffn(nc, P, prm, x_bf1, z)
    last = l == NL - 1
    if last:
        x2 = P["bigp"].tile([128, DC, OWN], F32, tag="h", name="x_final")
        _layernorm(nc, P, z, x2, None, prm["g2"], prm["be2"])
        return x2, None
    x2 = P["xp"].tile([128, DC, OWN], BF16, tag="x", name=f"x_ln2_{l}")
    x_all2 = P["xap"].tile([128, DC, S], F8, tag="xa", name=f"x_all_{l + 1}")
    _layernorm(nc, P, z, x2, x_all2, prm["g2"], prm["be2"])
    return x2, x_all2


def _tail(nc, P, dr, x_f32):
    psB = P["psB"]
    xout = dr["xout"]
    ident32 = P["constp"].tile([128, 128], F32, name="ident32")
    nc.vector.tensor_scalar_add(ident32[:], P["ident_sb"][:], 0.0)
    for ti in range(7):
        t0 = ti * 128
        tsz = min(128, OWN - t0)
        xo_sb = P["statp"].tile([128, D], F32, tag="st", name="xo_sb")
        for dt in range(DC):
            tp = psB.tile([128, 2, 512], F32, tag="s", name="tp")
            nc.tensor.transpose(
                tp[:tsz, 0, 0:128], x_f32[:, dt, t0 : t0 + tsz], ident32[:]
            )
            nc.vector.tensor_scalar_add(
                xo_sb[:tsz, dt * 128 : (dt + 1) * 128], tp[:tsz, 0, 0:128], 0.0
            )
        nc.sync.dma_start(xout[t0 : t0 + tsz, :], xo_sb[:tsz, :])


def _layernorm(nc, P, z, x_out, x_f8_out, g_sb, be_sb):
    """Post-LN over features (partition dim) in transposed layout."""
    psA, statp, zbp = P["psA"], P["statp"], P["zbp"]
    zf = z[:].bitcast(F32)
    sum_ps = psA.tile([128, 2, 512], F32, tag="o", name="sum_ps")
    for ci in range(2):
        for ct in range(DC):
            nc.tensor.matmul(
                sum_ps[:, ci, :HQ],
                P["ones_r"][:],
                z[:, ct, ci * HQ : (ci + 1) * HQ],
                start=(ct == 0),
                stop=(ct == DC - 1),
            )
    sq = zbp.tile([128, DC, OWN], BF16, tag="zb", name="sq_bf")
    for ct in range(DC):
        nc.vector.tensor_tensor(
            sq[:, ct, :], zf[:, ct, :], zf[:, ct, :], OP.mult
        )
    sq_ps = psA.tile([128, 2, 512], F32, tag="zz", name="sq_ps")
    for ci in range(2):
        for ct in range(DC):
            nc.tensor.matmul(
                sq_ps[:, ci, :HQ],
                P["ones_bf"][:],
                sq[:, ct, ci * HQ : (ci + 1) * HQ],
                start=(ct == 0),
                stop=(ct == DC - 1),
            )
    mu = statp.tile([128, 2, HQ], F32, tag="st", name="mu")
    nc.vector.tensor_scalar(
        mu[:], sum_ps[:, :, 0:HQ], 1.0 / D, None, OP.mult, OP.bypass
    )
    musq = statp.tile([128, 2, HQ], F32, tag="st", name="musq")
    nc.vector.tensor_tensor(musq[:], mu[:], mu[:], OP.mult)
    var = statp.tile([128, 2, HQ], F32, tag="st", name="var")
    nc.vector.scalar_tensor_tensor(
        var[:], sq_ps[:, :, 0:HQ], 1.0 / D, musq[:], OP.mult, OP.subtract
    )
    lnv = statp.tile([128, 2, HQ], F32, tag="st", name="lnv")
    nc.scalar.activation(lnv[:], var[:], AF.Ln, bias=P["eps_sb"][:])
    rstd = statp.tile([128, 2, HQ], F32, tag="st", name="rstd")
    nc.scalar.activation(rstd[:], lnv[:], AF.Exp, scale=-0.5)
    mr = statp.tile([128, 2, HQ], F32, tag="st", name="mr")
    nc.vector.tensor_tensor(mr[:], mu[:], rstd[:], OP.mult)
    rstd_f = rstd[:].rearrange("p a b -> p (a b)")
    mr_f = mr[:].rearrange("p a b -> p (a b)")
    for ct in range(DC):
        nc.vector.tensor_tensor(z[:, ct, :], zf[:, ct, :], rstd_f[:, :OWN], OP.mult)
        nc.vector.tensor_tensor(z[:, ct, :], zf[:, ct, :], mr_f[:, :OWN], OP.subtract)
        nc.scalar.activation(
            x_out[:, ct, :],
            zf[:, ct, :],
            AF.Identity,
            bias=be_sb[:, ct : ct + 1],
            scale=g_sb[:, ct : ct + 1],
        )
        if x_f8_out is not None:
            nc.scalar.activation(
                x_f8_out[:, ct, 0:OWN],
                zf[:, ct, :],
                AF.Identity,
                bias=be_sb[:, ct : ct + 1],
                scale=g_sb[:, ct : ct + 1],
            )


def _pow2_scale(absmax, target=224.0):
    if absmax <= 0:
        return 1.0
    return 2.0 ** math.floor(math.log2(target / absmax))


def _build_wvx(Wv, s_v):
    """s_v*Wv^T extended to [NL, D, NH*128]: per head a 64-col V block and
    a 64-col zero block; even heads [V|0], odd heads [0|V]."""
    f8 = ml_dtypes.float8_e4m3
    WvT = Wv.transpose(0, 2, 1)
    out = np.zeros((NL, D, NH * 128), np.float32)
    for h in range(NH):
        off = h * 128 + (0 if h % 2 == 0 else 64)
        out[:, :, off : off + 64] = WvT[:, :, h * 64 : (h + 1) * 64] * s_v
    return out.astype(f8)


def _build_bvx(bv, s_v):
    out = np.full((NL, NH * 128), s_v, np.float32)
    for h in range(NH):
        off = h * 128 + (0 if h % 2 == 0 else 64)
        out[:, off : off + 64] = bv[:, h * 64 : (h + 1) * 64] * s_v
    return out


_NC_CACHE = None
_EXP_SCALES = None
_SV_USED = None


def _host_prep(inputs):
    bf = ml_dtypes.bfloat16
    f8 = ml_dtypes.float8_e4m3
    vid = np.asarray(inputs["vid"], np.float32)
    x = vid.reshape(B, L, C, H // PH, PH, W // PW, PW)
    x = x.transpose(0, 1, 3, 5, 4, 6, 2).reshape(B, L, NP, PD)

    pos = np.asarray(inputs["pos_emb"], np.float32)[0]
    cls = np.asarray(inputs["cls"], np.float32)[0, :, 0, :]
    b_emb = np.asarray(inputs["b_embed"], np.float32)

    Wq = np.asarray(inputs["Wq"], np.float32)
    Wk = np.asarray(inputs["Wk"], np.float32)
    Wv = np.asarray(inputs["Wv"], np.float32)
    s_q = np.array([_pow2_scale(np.abs(Wq[l]).max()) for l in range(NL)])
    s_k = np.array([_pow2_scale(np.abs(Wk[l]).max()) for l in range(NL)])
    s_v = min(SV, min(_pow2_scale(np.abs(Wv[l]).max()) for l in range(NL)))
    global _EXP_SCALES, _SV_USED
    _EXP_SCALES = [float(0.125 / (s_q[l] * s_k[l])) for l in range(NL)]
    _SV_USED = float(s_v)

    def _dr_pack(wT):
        # [NL, c, d] -> [NL, p, c2, dt, parity, col] -> [NL, 128, 2048]
        a = wT.reshape(NL, 2, 2, 128, DC, 128)  # (c2, parity, p, dt, col)
        a = a.transpose(0, 3, 1, 4, 2, 5)  # (p, c2, dt, parity, col)
        return np.ascontiguousarray(a.reshape(NL, 128, 2 * DC * 2 * 128))

    wq8 = _dr_pack(np.ascontiguousarray(Wq.transpose(0, 2, 1))
                   * s_q[:, None, None])
    wk8 = _dr_pack(np.ascontiguousarray(Wk.transpose(0, 2, 1))
                   * s_k[:, None, None])

    shared = {
        "wembT": np.ascontiguousarray(
            np.asarray(inputs["W_embed"], np.float32).T
        ).astype(bf),
        "wqT8": wq8.astype(f8),
        "wkT8": wk8.astype(f8),
        "wvxT8": _build_wvx(Wv, s_v),
        "woT": np.ascontiguousarray(
            np.asarray(inputs["Wo"], np.float32).transpose(0, 2, 1)
        ).astype(bf),
        "w1T": np.ascontiguousarray(
            np.asarray(inputs["W1"], np.float32).transpose(0, 2, 1)
        ).astype(bf),
        "w2T": np.ascontiguousarray(
            np.asarray(inputs["W2"], np.float32).transpose(0, 2, 1)
        ).astype(bf),
        "bq": np.asarray(inputs["bq"], np.float32) * s_q[:, None],
        "bk": np.asarray(inputs["bk"], np.float32) * s_k[:, None],
        "bvx": _build_bvx(np.asarray(inputs["bv"], np.float32), s_v),
        "bo": np.asarray(inputs["bo"], np.float32),
        "b1": np.asarray(inputs["b1"], np.float32),
        "b2": np.asarray(inputs["b2"], np.float32),
        "g1": np.asarray(inputs["ln1_g"], np.float32),
        "be1": np.asarray(inputs["ln1_b"], np.float32),
        "g2": np.asarray(inputs["ln2_g"], np.float32),
        "be2": np.asarray(inputs["ln2_b"], np.float32),
        "ident": np.eye(128, dtype=np.float32).astype(bf),
        "swapid": np.roll(np.eye(128, dtype=np.float32), 64, axis=1),
    }

    in_maps = []
    for c in range(N_CORES):
        b, half = c // 2, c % 2
        f0 = half * (L // 2)
        pat_c = np.zeros((PD, OWN), np.float32)
        addv_c = np.zeros((D, OWN), np.float32)
        for f in range(L // 2):
            fr = f0 + f
            t0 = f * (NP + 1)
            pat_c[:, t0 + 1 : t0 + NP + 1] = x[b, fr].T
            addv_c[:, t0] = pos[fr, 0] + cls[fr]
            addv_c[:, t0 + 1 : t0 + NP + 1] = pos[fr, 1:].T + b_emb[:, None]
        m = {"pat": pat_c.astype(bf), "addv": addv_c}
        m.update(shared)
        in_maps.append(m)
    return in_maps


def kernel(**inputs):
    global _NC_CACHE
    in_maps = _host_prep(inputs)
    if _NC_CACHE is None:
        nc = build_kernel(_EXP_SCALES, _SV_USED)
        legalize_waits(nc)
        _NC_CACHE = nc
    nc = _NC_CACHE
    res = run_bass_kernel_spmd(nc, in_maps, core_ids=list(range(N_CORES)))
    out = np.zeros((B, S, D), np.float32)
    for c in range(N_CORES):
        b, half = c // 2, c % 2
        out[b, half * OWN : (half + 1) * OWN, :] = res.results[c]["xout"]
    return out


# revision 45
# speedup vs baseline: 4.1505x; 4.1505x over previous
"""Trainium2 Bass kernel for nn_Encoder_37340445671714 (video ViT encoder).

Sharding: 8 cores = 4 batch elements x 2 sequence halves (788 tokens each),
with a per-core LOCAL token order of [own 788 | partner 788] so the program
is identical on every core (SPMD).

Per layer:
  - pair exchange via ReduceScatter(add) of bf16 x (input duplicated): both
    cores receive the pair SUM; partner half = sum - own, written fp8 into
    x_all[:, :, OWN:]. The collective hides under own-half attention work.
  - Q/K/V projections and AV run in fp8e4 DoubleRow (2 contraction k-tiles
    per instruction); scores, Wo and the FFN stay bf16 (error budget).
  - flash attention, own-first k-tile order; one Exp per k-tile covering
    both heads of the pair (scores for both heads land in one 2-plane psum
    tile). V weights host-extended per head with a zero-weight/bias-sv
    block so AV also produces sv*Z (sv cancels in o = sv*num / (sv*Z));
    1/Z via DVE divide after an anti-diagonal f32r swap matmul. The ones
    columns of V' are memset once; per-tile V writes touch only V columns.
  - projection biases for Q/K-own applied on ScalarE (per-partition bias),
    FFN relu+bias on ScalarE; LayerNorm stats via ones-matmul partition
    sums (sumsq in bf16); LN2 also emits the fp8 x copy for the next layer.
Weight/scale prep happens on the host (free). Output transposed on the PE.
"""

import math

import numpy as np
import ml_dtypes

import concourse.bass as bass
import concourse.tile as tile
from concourse import mybir
from concourse.bass_utils import run_bass_kernel_spmd

F32 = mybir.dt.float32
F32R = mybir.dt.float32r
BF16 = mybir.dt.bfloat16
F8 = mybir.dt.float8e4
AF = mybir.ActivationFunctionType
OP = mybir.AluOpType
DR = mybir.MatmulPerfMode.DoubleRow
USE_DR = False

# problem dims
B, L, C, H, W = 4, 8, 3, 224, 224
PH = PW = 16
D = 512
NH = 8
DK = 64
FF = 2048
NL = 6
NP = (H // PH) * (W // PW)  # 196
S = L * (NP + 1)  # 1576
PD = PH * PW * C  # 768
OWN = S // 2  # 788 tokens per core
LN_EPS = 1e-5
SV = 32.0  # fp8 scale for extended V (cancels in softmax normalize)

DC = D // 128  # 4
PDC = PD // 128  # 6
FTC = FF // 128  # 16

KT = [(i * 128, 128) for i in range(S // 128)] + [(S - S % 128, S % 128)]
PAIRS = [(0, 1), (2, 3), (4, 5), (6, 7), (8, 9), (10, 11)]
SINGLE = 12
HQ = OWN // 2  # 394 (half of the own-token range; psum-bank-sized chunks)

N_CORES = 8
REPLICA_GROUPS = [[0, 1], [2, 3], [4, 5], [6, 7]]


def legalize_waits(nc):
    """Split multi-wait instructions into preceding single-wait NoOps."""
    n_split = 0
    for f in nc.m.functions:
        for bb in f.blocks:
            insts = list(bb.instructions)
            new_insts = []
            changed = False
            for inst in insts:
                si = inst.sync_info
                if si is not None and len(si.on_wait) > 1:
                    waits = list(si.on_wait)
                    for w in waits[:-1]:
                        nop = mybir.InstNoOp(
                            name=nc.get_next_instruction_name(),
                            engine=inst.engine,
                            ins=[],
                            outs=[],
                        )
                        nop.sync_info = mybir.SyncInfo(on_wait=[w], on_update=[])
                        new_insts.append(nop)
                        n_split += 1
                    inst.sync_info = mybir.SyncInfo(
                        on_wait=[waits[-1]], on_update=list(si.on_update)
                    )
                    changed = True
                new_insts.append(inst)
            if changed:
                bb.instructions = new_insts
    return n_split


def _bcast_ap(ap_1d, parts=128):
    return bass.AP(
        tensor=ap_1d.tensor, offset=ap_1d.offset, ap=[[0, parts]] + list(ap_1d.ap)
    )


def _vcols(base, ksz=None):
    """Two APs (even-head, odd-head) selecting the V columns (per 256
    lanes: [0,64) then [192,256)) of an AP whose last dim is [1, 1024]."""
    ap = [list(d) for d in base.ap]
    assert ap[-1][0] == 1 and ap[-1][1] == NH * 128
    p = ap[0]
    if ksz is not None:
        p = [p[0], ksz]
    return [
        bass.AP(tensor=base.tensor, offset=base.offset + off,
                ap=[p, [256, 4], [1, 64]])
        for off in (0, 192)
    ]


def _vcols_dr(v_f8, pi, j, ksz):
    """Two destination APs inside v_dr [128, pair, head, parity, 128]:
    even head h -> cols [0,64) of block h*256+j*128, odd -> [64,128)."""
    base = v_f8[:]
    pstr = base.ap[0][0]
    off = base.offset + pi * 2048 + j * 128
    return [
        bass.AP(tensor=base.tensor, offset=off + o2,
                ap=[[pstr, ksz], [512, 4], [1, 64]])
        for o2 in (0, 320)
    ]


def build_kernel(exp_scales, s_v):
    nc = bass.Bass(
        "TRN2", target_bir_lowering=False, debug=False, num_devices=N_CORES
    )

    pat = nc.dram_tensor("pat", [PD, OWN], BF16, kind="ExternalInput").ap()
    addv = nc.dram_tensor("addv", [D, OWN], F32, kind="ExternalInput").ap()
    wembT = nc.dram_tensor("wembT", [PD, D], BF16, kind="ExternalInput").ap()
    wqT = nc.dram_tensor("wqT8", [NL, 128, 2 * DC * 2 * 128], F8,
                         kind="ExternalInput").ap()
    wkT = nc.dram_tensor("wkT8", [NL, 128, 2 * DC * 2 * 128], F8,
                         kind="ExternalInput").ap()
    wvT = nc.dram_tensor("wvxT8", [NL, D, NH * 128], F8, kind="ExternalInput").ap()
    woT = nc.dram_tensor("woT", [NL, D, D], BF16, kind="ExternalInput").ap()
    w1T = nc.dram_tensor("w1T", [NL, D, FF], BF16, kind="ExternalInput").ap()
    w2T = nc.dram_tensor("w2T", [NL, FF, D], BF16, kind="ExternalInput").ap()
    bq = nc.dram_tensor("bq", [NL, D], F32, kind="ExternalInput").ap()
    bk = nc.dram_tensor("bk", [NL, D], F32, kind="ExternalInput").ap()
    bv = nc.dram_tensor("bvx", [NL, NH * 128], F32, kind="ExternalInput").ap()
    bo = nc.dram_tensor("bo", [NL, D], F32, kind="ExternalInput").ap()
    b1 = nc.dram_tensor("b1", [NL, FF], F32, kind="ExternalInput").ap()
    b2 = nc.dram_tensor("b2", [NL, D], F32, kind="ExternalInput").ap()
    g1 = nc.dram_tensor("g1", [NL, D], F32, kind="ExternalInput").ap()
    be1 = nc.dram_tensor("be1", [NL, D], F32, kind="ExternalInput").ap()
    g2 = nc.dram_tensor("g2", [NL, D], F32, kind="ExternalInput").ap()
    be2 = nc.dram_tensor("be2", [NL, D], F32, kind="ExternalInput").ap()
    ident = nc.dram_tensor("ident", [128, 128], BF16, kind="ExternalInput").ap()
    swapid = nc.dram_tensor("swapid", [128, 128], F32R, kind="ExternalInput").ap()
    xout = nc.dram_tensor("xout", [OWN, D], F32, kind="ExternalOutput").ap()

    with tile.TileContext(nc) as tc:
        with (
            tc.tile_pool(name="const", bufs=1) as constp,
            tc.tile_pool(name="wsmall", bufs=1) as wsmall,
            tc.tile_pool(name="wff", bufs=2) as wff,
            tc.tile_pool(name="xp", bufs=2) as xp,
            tc.tile_pool(name="xa", bufs=2) as xap,
            tc.tile_pool(name="kv", bufs=1) as kvp,
            tc.tile_pool(name="vp", bufs=1) as vp,
            tc.tile_pool(name="qo", bufs=1) as qop,
            tc.tile_pool(name="zp", bufs=1) as zp,
            tc.tile_pool(name="zbp", bufs=1) as zbp,
            tc.tile_pool(name="xs", bufs=1) as xsp,
            tc.tile_pool(name="big", bufs=1) as bigp,
            tc.tile_pool(name="exps", bufs=6) as expp,
            tc.tile_pool(name="stat", bufs=5) as statp,
            tc.tile_pool(name="rz", bufs=3) as rzp,
            tc.tile_pool(name="bias", bufs=2) as biasp,
            tc.tile_pool(name="psA", bufs=1, space="PSUM") as psA,
            tc.tile_pool(name="psB", bufs=2, space="PSUM") as psB,
            tc.tile_pool(name="dram", bufs=2, space="DRAM") as dramp,
        ):
            P = dict(
                constp=constp, wsmall=wsmall, wff=wff, xp=xp, xap=xap, kvp=kvp,
                vp=vp, qop=qop, zp=zp, zbp=zbp, xsp=xsp, bigp=bigp, expp=expp,
                statp=statp, rzp=rzp, biasp=biasp, psA=psA, psB=psB, dramp=dramp,
            )
            dram_in = dict(
                pat=pat, addv=addv, wembT=wembT, wqT=wqT, wkT=wkT, wvT=wvT,
                woT=woT, w1T=w1T, w2T=w2T, bq=bq, bk=bk, bv=bv, bo=bo, b1=b1,
                swapid=swapid, b2=b2, g1=g1, be1=be1, g2=g2, be2=be2,
                ident=ident, xout=xout,
            )
            ones_bf = constp.tile([128, 128], BF16, name="ones_bf")
            nc.vector.memset(ones_bf[:], 1.0)
            ones_r = constp.tile([128, 128], F32R, name="ones_r")
            nc.vector.tensor_scalar_add(ones_r[:], ones_bf[:], 0.0)
            ident_sb = constp.tile([128, 128], BF16, name="ident_sb")
            nc.sync.dma_start(ident_sb[:], ident[:])
            eps_sb = constp.tile([128, 1], F32, name="eps_sb")
            nc.vector.memset(eps_sb[:], LN_EPS)
            swap_sb = constp.tile([128, 128], F32R, name="swap_sb")
            nc.sync.dma_start(swap_sb[:], swapid[:])
            onesw = constp.tile([128, OWN], F32, name="onesw")
            nc.vector.memset(onesw[:], 1.0)
            P["ones_bf"] = ones_bf
            P["ones_r"] = ones_r
            P["ident_sb"] = ident_sb
            P["eps_sb"] = eps_sb
            P["swap_sb"] = swap_sb
            P["onesw"] = onesw

            # extended-V tiles live for the whole kernel; ones columns are
            # constant (= s_v) and written exactly once here.
            v_f8 = vp.tile([128, len(PAIRS), NH, 2, 128], F8, name="v_f8")
            v_12 = vp.tile([128, NH * 128], F8, name="v_12")
            P["v_f8"] = v_f8
            P["v_12"] = v_12
            vb = v_f8[:]
            pstr = vb.ap[0][0]
            # ones columns: even heads at h*256+64, odd heads at h*256 (+j*128)
            for base_off in (64, 256):
                nc.vector.memset(
                    bass.AP(
                        tensor=vb.tensor, offset=vb.offset + base_off,
                        ap=[[pstr, 128], [2048, len(PAIRS)], [512, 4],
                            [128, 2], [1, 64]],
                    ),
                    s_v,
                )
            v12b = v_12[:]
            nc.vector.memset(
                bass.AP(
                    tensor=v12b.tensor, offset=v12b.offset + 64,
                    ap=[[v12b.ap[0][0], 128], [256, 4], [1, 128]],
                ),
                s_v,
            )

            x_bf, x_all = _embed(nc, P, dram_in)
            for l in range(NL):
                x_bf, x_all = _one_layer(
                    nc, P, dram_in, l, x_bf, x_all, exp_scales[l]
                )
            _tail(nc, P, dram_in, x_bf)
    return nc


def _embed(nc, P, dr):
    bigp, zp, wff, xp, psB = P["bigp"], P["zp"], P["wff"], P["xp"], P["psB"]
    pat_sb = bigp.tile([128, FTC, OWN], BF16, tag="h", name="pat_sb")
    nc.sync.dma_start(
        pat_sb[:, :PDC, :], dr["pat"].rearrange("(ko p) t -> p ko t", p=128)
    )
    addv_sb = zp.tile([128, DC, OWN], F32, tag="z", name="addv_sb")
    nc.sync.dma_start(addv_sb[:], dr["addv"].rearrange("(co p) t -> p co t", p=128))
    wemb_sb = wff.tile([128, PDC, D], BF16, tag="wff", name="wemb_sb")
    nc.sync.dma_start(wemb_sb[:], dr["wembT"].rearrange("(ko p) d -> p ko d", p=128))

    x_bf = xp.tile([128, DC, OWN], BF16, tag="x", name="x_emb")
    x_all = P["xap"].tile([128, DC, S], F8, tag="xa", name="x_all_0")
    for dt in range(DC):
        ps = psB.tile([128, 2, 512], F32, tag="s", name="ps_emb")
        for ci in range(2):
            for kt in range(PDC):
                nc.tensor.matmul(
                    ps[:, ci, :HQ],
                    wemb_sb[:, kt, dt * 128 : (dt + 1) * 128],
                    pat_sb[:, kt, ci * HQ : (ci + 1) * HQ],
                    start=(kt == 0),
                    stop=(kt == PDC - 1),
                )
        nc.vector.tensor_tensor(
            x_bf[:, dt, :].rearrange("p (a b) -> p a b", a=2),
            ps[:, :, :HQ],
            addv_sb[:, dt, :].rearrange("p (a b) -> p a b", a=2),
            OP.add,
        )
        nc.vector.tensor_tensor(
            x_all[:, dt, 0:OWN].rearrange("p (a b) -> p a b", a=2),
            ps[:, :, :HQ],
            addv_sb[:, dt, :].rearrange("p (a b) -> p a b", a=2),
            OP.add,
        )
    return x_bf, x_all


def _load_layer_params(nc, P, dr, l):
    biasp, wsmall, wff = P["biasp"], P["wsmall"], P["wff"]
    prm = {}
    for nm in ["bq", "bk", "bo", "b2", "g1", "be1", "g2", "be2"]:
        t = biasp.tile([128, DC], F32, tag=nm, name=nm + "_sb")
        nc.sync.dma_start(t[:], dr[nm][l].rearrange("(o p) -> p o", p=128))
        prm[nm] = t
    b1_sb = biasp.tile([128, FTC], F32, tag="b1", name="b1_sb")
    nc.sync.dma_start(b1_sb[:], dr["b1"][l].rearrange("(o p) -> p o", p=128))
    prm["b1"] = b1_sb
    bv_bc = biasp.tile([128, NH * 128], BF16, tag="bvb", name="bv_bc")
    nc.gpsimd.dma_start(bv_bc[:], _bcast_ap(dr["bv"][l]))
    prm["bv_bc"] = bv_bc
    for nm, key in [("wq", "wqT"), ("wk", "wkT")]:
        t = wsmall.tile([128, 2, DC, 2, 128], F8, tag=nm, name=nm + "_sb")
        nc.sync.dma_start(
            t[:].rearrange("p a b c d -> p (a b c d)"), dr[key][l]
        )
        prm[nm] = t
    wo = wsmall.tile([128, DC, D], BF16, tag="wo", name="wo_sb")
    nc.sync.dma_start(wo[:], dr["woT"][l].rearrange("(co p) d -> p co d", p=128))
    prm["wo"] = wo
    wv = wsmall.tile([128, DC, NH * 128], F8, tag="wv", name="wv_sb")
    nc.sync.dma_start(wv[:], dr["wvT"][l].rearrange("(co p) d -> p co d", p=128))
    prm["wv"] = wv
    w1_sb = wff.tile([128, DC, FF], BF16, tag="wff", name="w1_sb")
    nc.sync.dma_start(w1_sb[:], dr["w1T"][l].rearrange("(co p) f -> p co f", p=128))
    prm["w1"] = w1_sb
    w2_sb = wff.tile([128, FTC, D], BF16, tag="wff", name="w2_sb")
    nc.sync.dma_start(w2_sb[:], dr["w2T"][l].rearrange("(fo p) d -> p fo d", p=128))
    prm["w2"] = w2_sb
    return prm


def _exchange_start(nc, P, x_bf):
    dramp = P["dramp"]
    xg_in = dramp.tile([2, DC, 128, OWN], BF16, tag="agi", name="xg_in")
    for s_ in range(2):
        nc.sync.dma_start(xg_in[s_].rearrange("c p t -> p c t"), x_bf[:])
    xg_sum = dramp.tile([DC, 128, OWN], BF16, tag="ago", name="xg_sum")
    nc.gpsimd.collective_compute(
        "ReduceScatter",
        OP.add,
        replica_groups=REPLICA_GROUPS,
        ins=[xg_in[:].opt()],
        outs=[xg_sum[:].opt()],
    )
    return xg_sum


def _exchange_finish(nc, P, xg_sum, x_bf, x_all):
    xsum_sb = P["xsp"].tile([128, DC, OWN], BF16, tag="xs", name="xsum_sb")
    nc.sync.dma_start(xsum_sb[:], xg_sum.rearrange("c p t -> p c t"))
    for h in range(2):
        cs = slice(2 * h, 2 * h + 2)
        nc.vector.tensor_tensor(
            x_all[:, cs, OWN:S], xsum_sb[:, cs, :], x_bf[:, cs, :], OP.subtract
        )


def _proj_dr(nc, P, w_f8, x_src, out_sb, bias_sb, t0, tn, act_bias):
    """out_sb[:, dt, t0:t0+tn] (bf16) = fp8 DoubleRow proj + bias."""
    psB = P["psB"]
    hn = tn // 2
    for dt in range(DC):
        ps = psB.tile([128, 2, 512], F32, tag="s", name="ps_p")
        for ci in range(2):
            q0 = t0 + ci * hn
            if USE_DR:
                for c2 in range(2):
                    nc.tensor.matmul(
                        ps[:, ci, :hn],
                        w_f8[:, c2, dt, :, :],
                        x_src[:, 2 * c2 : 2 * c2 + 2, q0 : q0 + hn],
                        start=(c2 == 0),
                        stop=(c2 == 1),
                        perf_mode=DR,
                    )
            else:
                for ct in range(DC):
                    nc.tensor.matmul(
                        ps[:, ci, :hn],
                        w_f8[:, ct // 2, dt, ct % 2, :],
                        x_src[:, ct, q0 : q0 + hn],
                        start=(ct == 0),
                        stop=(ct == DC - 1),
                    )
        dst = out_sb[:, dt, t0 : t0 + tn].rearrange("p (a b) -> p a b", a=2)
        if act_bias:
            nc.scalar.activation(
                dst, ps[:, :, :hn], AF.Identity, bias=bias_sb[:, dt : dt + 1]
            )
        else:
            nc.vector.tensor_scalar_add(dst, ps[:, :, :hn], bias_sb[:, dt : dt + 1])


def _v_proj_tile(nc, P, prm, x_all, t, v_dst_cols, ps_pool_tag):
    """V columns of the extended-V tile for k-tile t (+ bias), fp8 out."""
    k0, ksz = KT[t]
    if ps_pool_tag == "s":
        pst = P["psB"].tile([128, 2, 512], F32, tag="s", name="ps_v")
    else:
        pst = P["psA"].tile([128, 2, 512], F32, tag=ps_pool_tag, name="ps_v")
    for half in range(2):
        for ct in range(DC):
            nc.tensor.matmul(
                pst[:ksz, half, :],
                x_all[:, ct, k0 : k0 + ksz],
                prm["wv"][:, ct, half * 512 : (half + 1) * 512],
                start=(ct == 0),
                stop=(ct == DC - 1),
            )
    ps_flat = bass.AP(
        tensor=pst[:].tensor, offset=pst[:].offset,
        ap=[list(pst[:].ap[0])] + [[1, 1024]],
    )
    for dst, src, bv in zip(
        v_dst_cols, _vcols(ps_flat, ksz), _vcols(prm["bv_bc"][:], ksz)
    ):
        nc.vector.tensor_tensor(dst, src, bv, OP.add)


def _flash_pairs(nc, P, prm, kT, qT, p1, p2, sc_exp, hpair, pis, first_pi):
    psB, expp = P["psB"], P["expp"]
    hdt = hpair
    v_f8 = P["v_f8"]
    for pi in pis:
        ta, tb = PAIRS[pi]
        for sub in range(2):
            hp = sub * 64
            head = 2 * hpair + sub
            e_t = expp.tile([128, 2, 2, HQ], F8, tag="e", name="e_t")
            for j, t in enumerate((ta, tb)):
                k0, ksz = KT[t]
                s_ps = psB.tile([128, 2, 512], F32, tag="s", name="s_ps")
                for ci in range(2):
                    nc.tensor.matmul(
                        s_ps[:ksz, ci, :HQ],
                        kT[hp : hp + 64, hdt, k0 : k0 + ksz],
                        qT[hp : hp + 64, hdt, ci * HQ : (ci + 1) * HQ],
                        start=True,
                        stop=True,
                    )
                nc.scalar.activation(
                    e_t[:ksz, :, j, :], s_ps[:ksz, :, 0:HQ], AF.Exp, scale=sc_exp
                )
            dst = p1 if sub == 0 else p2
            for ci in range(2):
                if USE_DR:
                    nc.tensor.matmul(
                        dst[:, ci, :HQ],
                        v_f8[:, pi, head, :, :],
                        e_t[:, ci, :, :],
                        start=(pi == first_pi),
                        stop=False,
                        perf_mode=DR,
                    )
                else:
                    for j in range(2):
                        nc.tensor.matmul(
                            dst[:, ci, :HQ],
                            v_f8[:, pi, head, j, :],
                            e_t[:, ci, j, :],
                            start=(pi == first_pi and j == 0),
                            stop=False,
                        )


def _flash_single_and_rz(nc, P, prm, kT, qT, p1, p2, sc_exp, hpair, oT):
    psB, expp, rzp = P["psB"], P["expp"], P["rzp"]
    hdt = hpair
    k0, ksz = KT[SINGLE]
    for sub in range(2):
        hp = sub * 64
        head = 2 * hpair + sub
        e_t = expp.tile([128, 2, 2, HQ], F8, tag="e", name="e_t12")
        s_ps = psB.tile([128, 2, 512], F32, tag="s", name="s_p12")
        for ci in range(2):
            nc.tensor.matmul(
                s_ps[:ksz, ci, :HQ],
                kT[hp : hp + 64, hdt, k0 : k0 + ksz],
                qT[hp : hp + 64, hdt, ci * HQ : (ci + 1) * HQ],
                start=True,
                stop=True,
            )
        nc.scalar.activation(
            e_t[:ksz, :, 0, :], s_ps[:ksz, :, 0:HQ], AF.Exp, scale=sc_exp
        )
        dst = p1 if sub == 0 else p2
        for ci in range(2):
            nc.tensor.matmul(
                dst[:, ci, :HQ],
                P["v_12"][:ksz, head * 128 : (head + 1) * 128],
                e_t[:ksz, ci, 0, :],
                start=False,
                stop=True,
            )
    # 1/Z: Z_o in p2[0:64], Z_e in p1[64:128]; ln+exp base-aligned, then
    # swap halves with the f32r anti-diagonal-identity matmul.
    lnmix = rzp.tile([128, 2, HQ], F32, tag="rz", name="lnmix")
    nc.scalar.activation(lnmix[0:64, :, :], p2[0:64, :, 0:HQ], AF.Ln)
    nc.scalar.activation(lnmix[64:128, :, :], p1[64:128, :, 0:HQ], AF.Ln)
    rzmix = rzp.tile([128, 2, HQ], F32R, tag="rz", name="rzmix")
    nc.scalar.activation(rzmix[:], lnmix[:], AF.Exp, scale=-1.0)
    rsw = psB.tile([128, 2, 512], F32, tag="s", name="rsw")
    for ci in range(2):
        nc.tensor.matmul(
            rsw[:, ci, :HQ], P["swap_sb"][:], rzmix[:, ci, :], start=True,
            stop=True,
        )
    rzs = rzp.tile([128, 2, HQ], F32, tag="rz", name="rzs")
    nc.vector.tensor_scalar_add(rzs[:], rsw[:, :, 0:HQ], 0.0)
    nc.vector.tensor_tensor(
        oT[0:64, hdt, :].rearrange("p (a b) -> p a b", a=2),
        p1[0:64, :, 0:HQ], rzs[0:64, :, :], OP.mult,
    )
    nc.vector.tensor_tensor(
        oT[64:128, hdt, :].rearrange("p (a b) -> p a b", a=2),
        p2[64:128, :, 0:HQ], rzs[64:128, :, :], OP.mult,
    )


def _attention(nc, P, prm, x_all, sc_exp, xg_sum, x_bf):
    psA, qop = P["psA"], P["qop"]
    qT = qop.tile([128, DC, OWN], BF16, tag="qT", name="qT")
    _proj_dr(nc, P, prm["wq"], x_all, qT, prm["bq"], 0, OWN, act_bias=True)
    kT = P["kvp"].tile([128, DC, S], BF16, tag="kv", name="kT")
    _proj_dr(nc, P, prm["wk"], x_all, kT, prm["bk"], 0, OWN, act_bias=True)

    v_f8 = P["v_f8"]
    for pi in range(3):
        for j in range(2):
            t = PAIRS[pi][j]
            _v_proj_tile(
                nc, P, prm, x_all, t, _vcols_dr(v_f8, pi, j, KT[t][1]),
                "o" if (2 * pi + j) % 2 == 0 else "zz",
            )

    oT = qop.tile([128, DC, OWN], BF16, tag="oT", name="oT")
    # hpair 0: own pairs first (the exchange is still in flight)
    p1 = psA.tile([128, 2, 512], F32, tag="o", name="p1_0")
    p2 = psA.tile([128, 2, 512], F32, tag="zz", name="p2_0")
    _flash_pairs(nc, P, prm, kT, qT, p1, p2, sc_exp, 0, range(0, 3), 0)

    # partner half lands; finish exchange, late K/V
    _exchange_finish(nc, P, xg_sum, x_bf, x_all)
    _proj_dr(nc, P, prm["wk"], x_all, kT, prm["bk"], OWN, OWN, act_bias=False)
    for pi in range(3, len(PAIRS)):
        for j in range(2):
            t = PAIRS[pi][j]
            _v_proj_tile(
                nc, P, prm, x_all, t, _vcols_dr(v_f8, pi, j, KT[t][1]), "s"
            )
    _v_proj_tile(
        nc, P, prm, x_all, SINGLE, _vcols(P["v_12"][:], KT[SINGLE][1]), "s"
    )

    _flash_pairs(nc, P, prm, kT, qT, p1, p2, sc_exp, 0, range(3, len(PAIRS)), 0)
    _flash_single_and_rz(nc, P, prm, kT, qT, p1, p2, sc_exp, 0, oT)
    for hpair in range(1, NH // 2):
        p1 = psA.tile([128, 2, 512], F32, tag="o", name=f"p1_{hpair}")
        p2 = psA.tile([128, 2, 512], F32, tag="zz", name=f"p2_{hpair}")
        _flash_pairs(
            nc, P, prm, kT, qT, p1, p2, sc_exp, hpair, range(len(PAIRS)), 0
        )
        _flash_single_and_rz(nc, P, prm, kT, qT, p1, p2, sc_exp, hpair, oT)
    return oT


def _residual_proj(nc, P, w_sb, rhs_T, bias_sb, x_bf, z_out):
    """z_out (f32) = bf16 W^T proj of rhs_T + bias + x_bf (residual)."""
    psB = P["psB"]
    for dt in range(DC):
        ps = psB.tile([128, 2, 512], F32, tag="s", name="ps_r")
        for ci in range(2):
            for ct in range(DC):
                nc.tensor.matmul(
                    ps[:, ci, :HQ],
                    w_sb[:, ct, dt * 128 : (dt + 1) * 128],
                    rhs_T[:, ct, ci * HQ : (ci + 1) * HQ],
                    start=(ct == 0),
                    stop=(ct == DC - 1),
                )
        nc.vector.scalar_tensor_tensor(
            z_out[:, dt, :].rearrange("p (a b) -> p a b", a=2),
            ps[:, :, :HQ],
            bias_sb[:, dt : dt + 1],
            x_bf[:, dt, :].rearrange("p (a b) -> p a b", a=2),
            OP.add,
            OP.add,
        )


def _ffn(nc, P, prm, x_bf, z_out):
    psA, bigp = P["psA"], P["bigp"]
    h_bf = bigp.tile([128, FTC, OWN], BF16, tag="h", name="h_bf")
    for ft in range(FTC):
        ps = psA.tile(
            [128, 2, 512], F32, tag=("o" if ft % 2 == 0 else "zz"), name="ps_h"
        )
        for ci in range(2):
            for ct in range(DC):
                nc.tensor.matmul(
                    ps[:, ci, :HQ],
                    prm["w1"][:, ct, ft * 128 : (ft + 1) * 128],
                    x_bf[:, ct, ci * HQ : (ci + 1) * HQ],
                    start=(ct == 0),
                    stop=(ct == DC - 1),
                )
        nc.scalar.activation(
            h_bf[:, ft, :].rearrange("p (a b) -> p a b", a=2),
            ps[:, :, :HQ],
            AF.Relu,
            bias=prm["b1"][:, ft : ft + 1],
        )
    for dt in range(DC):
        ps2 = psA.tile(
            [128, 2, 512], F32, tag=("o" if dt % 2 == 0 else "zz"), name="ps_f"
        )
        for ci in range(2):
            for ft in range(FTC):
                nc.tensor.matmul(
                    ps2[:, ci, :HQ],
                    prm["w2"][:, ft, dt * 128 : (dt + 1) * 128],
                    h_bf[:, ft, ci * HQ : (ci + 1) * HQ],
                    start=(ft == 0),
                    stop=(ft == FTC - 1),
                )
        nc.vector.scalar_tensor_tensor(
            z_out[:, dt, :].rearrange("p (a b) -> p a b", a=2),
            ps2[:, :, :HQ],
            prm["b2"][:, dt : dt + 1],
            x_bf[:, dt, :].rearrange("p (a b) -> p a b", a=2),
            OP.add,
            OP.add,
        )


def _one_layer(nc, P, dr, l, x_bf, x_all, sc_exp):
    xg_sum = _exchange_start(nc, P, x_bf)
    prm = _load_layer_params(nc, P, dr, l)
    oT = _attention(nc, P, prm, x_all, sc_exp, xg_sum, x_bf)
    z = P["zp"].tile([128, DC, OWN], F32R, tag="z", name=f"z1_{l}")
    _residual_proj(nc, P, prm["wo"], oT, prm["bo"], x_bf, z)
    x_bf1 = P["xp"].tile([128, DC, OWN], BF16, tag="x", name=f"x_ln1_{l}")
    _layernorm(nc, P, z, x_bf1, None, prm["g1"], prm["be1"])
    z = P["zp"].tile([128, DC, OWN], F32R, tag="z", name=f"z2_{l}")
    _ffn(nc, P, prm, x_bf1, z)
    last = l == NL - 1
    if last:
        x2 = P["bigp"].tile([128, DC, OWN], F32, tag="h", name="x_final")
        _layernorm(nc, P, z, x2, None, prm["g2"], prm["be2"])
        return x2, None
    x2 = P["xp"].tile([128, DC, OWN], BF16, tag="x", name=f"x_ln2_{l}")
    x_all2 = P["xap"].tile([128, DC, S], F8, tag="xa", name=f"x_all_{l + 1}")
    _layernorm(nc, P, z, x2, x_all2, prm["g2"], prm["be2"])
    return x2, x_all2


def _tail(nc, P, dr, x_f32):
    psB = P["psB"]
    xout = dr["xout"]
    ident32 = P["constp"].tile([128, 128], F32, name="ident32")
    nc.vector.tensor_scalar_add(ident32[:], P["ident_sb"][:], 0.0)
    for ti in range(7):
        t0 = ti * 128
        tsz = min(128, OWN - t0)
        xo_sb = P["statp"].tile([128, D], F32, tag="st", name="xo_sb")
        for dt in range(DC):
            tp = psB.tile([128, 2, 512], F32, tag="s", name="tp")
            nc.tensor.transpose(
                tp[:tsz, 0, 0:128], x_f32[:, dt, t0 : t0 + tsz], ident32[:]
            )
            nc.vector.tensor_scalar_add(
                xo_sb[:tsz, dt * 128 : (dt + 1) * 128], tp[:tsz, 0, 0:128], 0.0
            )
        nc.sync.dma_start(xout[t0 : t0 + tsz, :], xo_sb[:tsz, :])


def _layernorm(nc, P, z, x_out, x_f8_out, g_sb, be_sb):
    """Post-LN over features (partition dim) in transposed layout."""
    psA, statp, zbp = P["psA"], P["statp"], P["zbp"]
    zf = z[:].bitcast(F32)
    sum_ps = psA.tile([128, 2, 512], F32, tag="o", name="sum_ps")
    for ci in range(2):
        for ct in range(DC):
            nc.tensor.matmul(
                sum_ps[:, ci, :HQ],
                P["ones_r"][:],
                z[:, ct, ci * HQ : (ci + 1) * HQ],
                start=(ct == 0),
                stop=(ct == DC - 1),
            )
    sq = zbp.tile([128, DC, OWN], BF16, tag="zb", name="sq_bf")
    for ct in range(DC):
        nc.vector.tensor_tensor(
            sq[:, ct, :], zf[:, ct, :], zf[:, ct, :], OP.mult
        )
    sq_ps = psA.tile([128, 2, 512], F32, tag="zz", name="sq_ps")
    for ci in range(2):
        for ct in range(DC):
            nc.tensor.matmul(
                sq_ps[:, ci, :HQ],
                P["ones_bf"][:],
                sq[:, ct, ci * HQ : (ci + 1) * HQ],
                start=(ct == 0),
                stop=(ct == DC - 1),
            )
    mu = statp.tile([128, 2, HQ], F32, tag="st", name="mu")
    nc.vector.tensor_scalar(
        mu[:], sum_ps[:, :, 0:HQ], 1.0 / D, None, OP.mult, OP.bypass
    )
    musq = statp.tile([128, 2, HQ], F32, tag="st", name="musq")
    nc.vector.tensor_tensor(musq[:], mu[:], mu[:], OP.mult)
    var = statp.tile([128, 2, HQ], F32, tag="st", name="var")
    nc.vector.scalar_tensor_tensor(
        var[:], sq_ps[:, :, 0:HQ], 1.0 / D, musq[:], OP.mult, OP.subtract
    )
    lnv = statp.tile([128, 2, HQ], F32, tag="st", name="lnv")
    nc.scalar.activation(lnv[:], var[:], AF.Ln, bias=P["eps_sb"][:])
    rstd = statp.tile([128, 2, HQ], F32, tag="st", name="rstd")
    nc.scalar.activation(rstd[:], lnv[:], AF.Exp, scale=-0.5)
    mr = statp.tile([128, 2, HQ], F32, tag="st", name="mr")
    nc.vector.tensor_tensor(mr[:], mu[:], rstd[:], OP.mult)
    rstd_f = rstd[:].rearrange("p a b -> p (a b)")
    mr_f = mr[:].rearrange("p a b -> p (a b)")
    for ct in range(DC):
        nc.vector.tensor_tensor(z[:, ct, :], zf[:, ct, :], rstd_f[:, :OWN], OP.mult)
        nc.vector.tensor_tensor(z[:, ct, :], zf[:, ct, :], mr_f[:, :OWN], OP.subtract)
        nc.scalar.activation(
            x_out[:, ct, :],
            zf[:, ct, :],
            AF.Identity,
            bias=be_sb[:, ct : ct + 1],
            scale=g_sb[:, ct : ct + 1],
        )
        if x_f8_out is not None:
            nc.scalar.activation(
                x_f8_out[:, ct, 0:OWN],
                zf[:, ct, :],
                AF.Identity,
                bias=be_sb[:, ct : ct + 1],
                scale=g_sb[:, ct : ct + 1],
            )


def _pow2_scale(absmax, target=224.0):
    if absmax <= 0:
        return 1.0
    return 2.0 ** math.floor(math.log2(target / absmax))


def _build_wvx(Wv, s_v):
    """s_v*Wv^T extended to [NL, D, NH*128]: per head a 64-col V block and
    a 64-col zero block; even heads [V|0], odd heads [0|V]."""
    f8 = ml_dtypes.float8_e4m3
    WvT = Wv.transpose(0, 2, 1)
    out = np.zeros((NL, D, NH * 128), np.float32)
    for h in range(NH):
        off = h * 128 + (0 if h % 2 == 0 else 64)
        out[:, :, off : off + 64] = WvT[:, :, h * 64 : (h + 1) * 64] * s_v
    return out.astype(f8)


def _build_bvx(bv, s_v):
    out = np.full((NL, NH * 128), s_v, np.float32)
    for h in range(NH):
        off = h * 128 + (0 if h % 2 == 0 else 64)
        out[:, off : off + 64] = bv[:, h * 64 : (h + 1) * 64] * s_v
    return out


_NC_CACHE = None
_EXP_SCALES = None
_SV_USED = None


def _host_prep(inputs):
    bf = ml_dtypes.bfloat16
    f8 = ml_dtypes.float8_e4m3
    vid = np.asarray(inputs["vid"], np.float32)
    x = vid.reshape(B, L, C, H // PH, PH, W // PW, PW)
    x = x.transpose(0, 1, 3, 5, 4, 6, 2).reshape(B, L, NP, PD)

    pos = np.asarray(inputs["pos_emb"], np.float32)[0]
    cls = np.asarray(inputs["cls"], np.float32)[0, :, 0, :]
    b_emb = np.asarray(inputs["b_embed"], np.float32)

    Wq = np.asarray(inputs["Wq"], np.float32)
    Wk = np.asarray(inputs["Wk"], np.float32)
    Wv = np.asarray(inputs["Wv"], np.float32)
    s_q = np.array([_pow2_scale(np.abs(Wq[l]).max()) for l in range(NL)])
    s_k = np.array([_pow2_scale(np.abs(Wk[l]).max()) for l in range(NL)])
    s_v = min(SV, min(_pow2_scale(np.abs(Wv[l]).max()) for l in range(NL)))
    global _EXP_SCALES, _SV_USED
    _EXP_SCALES = [float(0.125 / (s_q[l] * s_k[l])) for l in range(NL)]
    _SV_USED = float(s_v)

    def _dr_pack(wT):
        # [NL, c, d] -> [NL, p, c2, dt, parity, col] -> [NL, 128, 2048]
        a = wT.reshape(NL, 2, 2, 128, DC, 128)  # (c2, parity, p, dt, col)
        a = a.transpose(0, 3, 1, 4, 2, 5)  # (p, c2, dt, parity, col)
        return np.ascontiguousarray(a.reshape(NL, 128, 2 * DC * 2 * 128))

    wq8 = _dr_pack(np.ascontiguousarray(Wq.transpose(0, 2, 1))
                   * s_q[:, None, None])
    wk8 = _dr_pack(np.ascontiguousarray(Wk.transpose(0, 2, 1))
                   * s_k[:, None, None])

    shared = {
        "wembT": np.ascontiguousarray(
            np.asarray(inputs["W_embed"], np.float32).T
        ).astype(bf),
        "wqT8": wq8.astype(f8),
        "wkT8": wk8.astype(f8),
        "wvxT8": _build_wvx(Wv, s_v),
        "woT": np.ascontiguousarray(
            np.asarray(inputs["Wo"], np.float32).transpose(0, 2, 1)
        ).astype(bf),
        "w1T": np.ascontiguousarray(
            np.asarray(inputs["W1"], np.float32).transpose(0, 2, 1)
        ).astype(bf),
        "w2T": np.ascontiguousarray(
            np.asarray(inputs["W2"], np.float32).transpose(0, 2, 1)
        ).astype(bf),
        "bq": np.asarray(inputs["bq"], np.float32) * s_q[:, None],
        "bk": np.asarray(inputs["bk"], np.float32) * s_k[:, None],
        "bvx": _build_bvx(np.asarray(inputs["bv"], np.float32), s_v),
        "bo": np.asarray(inputs["bo"], np.float32),
        "b1": np.asarray(inputs["b1"], np.float32),
        "b2": np.asarray(inputs["b2"], np.float32),
        "g1": np.asarray(inputs["ln1_g"], np.float32),
        "be1": np.asarray(inputs["ln1_b"], np.float32),
        "g2": np.asarray(inputs["ln2_g"], np.float32),
        "be2": np.asarray(inputs["ln2_b"], np.float32),
        "ident": np.eye(128, dtype=np.float32).astype(bf),
        "swapid": np.roll(np.eye(128, dtype=np.float32), 64, axis=1),
    }

    in_maps = []
    for c in range(N_CORES):
        b, half = c // 2, c % 2
        f0 = half * (L // 2)
        pat_c = np.zeros((PD, OWN), np.float32)
        addv_c = np.zeros((D, OWN), np.float32)
        for f in range(L // 2):
            fr = f0 + f
            t0 = f * (NP + 1)
            pat_c[:, t0 + 1 : t0 + NP + 1] = x[b, fr].T
            addv_c[:, t0] = pos[fr, 0] + cls[fr]
            addv_c[:, t0 + 1 : t0 + NP + 1] = pos[fr, 1:].T + b_emb[:, None]
        m = {"pat": pat_c.astype(bf), "addv": addv_c}
        m.update(shared)
        in_maps.append(m)
    return in_maps


def kernel(**inputs):
    global _NC_CACHE
    in_maps = _host_prep(inputs)
    if _NC_CACHE is None:
        nc = build_kernel(_EXP_SCALES, _SV_USED)
        legalize_waits(nc)
        _NC_CACHE = nc
    nc = _NC_CACHE
    res = run_bass_kernel_spmd(nc, in_maps, core_ids=list(range(N_CORES)))
    out = np.zeros((B, S, D), np.float32)
    for c in range(N_CORES):
        b, half = c // 2, c % 2
        out[b, half * OWN : (half + 1) * OWN, :] = res.results[c]["xout"]
    return out


# revision 53
# speedup vs baseline: 9.3557x; 2.2541x over previous
"""Trainium2 Bass kernel for nn_Encoder_37340445671714 (video ViT encoder).

Sharding: 8 cores = 4 batch elements x 2 sequence halves (788 tokens each),
with a per-core LOCAL token order of [own 788 | partner 788] so the program
is identical on every core (SPMD).

Per layer:
  - pair exchange via ReduceScatter(add) of bf16 x (input duplicated): both
    cores receive the pair SUM; partner half = sum - own, written fp8 into
    x_all[:, :, OWN:]. The collective hides under own-half attention work.
  - Q/K/V projections and AV run in fp8e4 DoubleRow (2 contraction k-tiles
    per instruction); scores, Wo and the FFN stay bf16 (error budget).
  - flash attention, own-first k-tile order; one Exp per k-tile covering
    both heads of the pair (scores for both heads land in one 2-plane psum
    tile). V weights host-extended per head with a zero-weight/bias-sv
    block so AV also produces sv*Z (sv cancels in o = sv*num / (sv*Z));
    1/Z via DVE divide after an anti-diagonal f32r swap matmul. The ones
    columns of V' are memset once; per-tile V writes touch only V columns.
  - projection biases for Q/K-own applied on ScalarE (per-partition bias),
    FFN relu+bias on ScalarE; LayerNorm stats via ones-matmul partition
    sums (sumsq in bf16); LN2 also emits the fp8 x copy for the next layer.
Weight/scale prep happens on the host (free). Output transposed on the PE.
"""

import math

import numpy as np
import ml_dtypes

import concourse.bass as bass
import concourse.tile as tile
from concourse import mybir
from concourse.bass_utils import run_bass_kernel_spmd

F32 = mybir.dt.float32
F32R = mybir.dt.float32r
BF16 = mybir.dt.bfloat16
F8 = mybir.dt.float8e4
AF = mybir.ActivationFunctionType
OP = mybir.AluOpType
DR = mybir.MatmulPerfMode.DoubleRow
USE_DR = False

# problem dims
B, L, C, H, W = 4, 8, 3, 224, 224
PH = PW = 16
D = 512
NH = 8
DK = 64
FF = 2048
NL = 6
NP = (H // PH) * (W // PW)  # 196
S = L * (NP + 1)  # 1576
PD = PH * PW * C  # 768
OWN = S // 2  # 788 tokens per core
LN_EPS = 1e-5
SV = 32.0  # fp8 scale for extended V (cancels in softmax normalize)

DC = D // 128  # 4
PDC = PD // 128  # 6
FTC = FF // 128  # 16

KT = [(i * 128, 128) for i in range(S // 128)] + [(S - S % 128, S % 128)]
PAIRS = [(0, 1), (2, 3), (4, 5), (6, 7), (8, 9), (10, 11)]
SINGLE = 12
HQ = OWN // 2  # 394 (half of the own-token range; psum-bank-sized chunks)

N_CORES = 8
REPLICA_GROUPS = [[0, 1], [2, 3], [4, 5], [6, 7]]


def legalize_waits(nc):
    """Split multi-wait instructions into preceding single-wait NoOps."""
    n_split = 0
    for f in nc.m.functions:
        for bb in f.blocks:
            insts = list(bb.instructions)
            new_insts = []
            changed = False
            for inst in insts:
                si = inst.sync_info
                if si is not None and len(si.on_wait) > 1:
                    waits = list(si.on_wait)
                    for w in waits[:-1]:
                        nop = mybir.InstNoOp(
                            name=nc.get_next_instruction_name(),
                            engine=inst.engine,
                            ins=[],
                            outs=[],
                        )
                        nop.sync_info = mybir.SyncInfo(on_wait=[w], on_update=[])
                        new_insts.append(nop)
                        n_split += 1
                    inst.sync_info = mybir.SyncInfo(
                        on_wait=[waits[-1]], on_update=list(si.on_update)
                    )
                    changed = True
                new_insts.append(inst)
            if changed:
                bb.instructions = new_insts
    return n_split


def _bcast_ap(ap_1d, parts=128):
    return bass.AP(
        tensor=ap_1d.tensor, offset=ap_1d.offset, ap=[[0, parts]] + list(ap_1d.ap)
    )


def _vcols(base, ksz=None):
    """Two APs (even-head, odd-head) selecting the V columns (per 256
    lanes: [0,64) then [192,256)) of an AP whose last dim is [1, 1024]."""
    ap = [list(d) for d in base.ap]
    assert ap[-1][0] == 1 and ap[-1][1] == NH * 128
    p = ap[0]
    if ksz is not None:
        p = [p[0], ksz]
    return [
        bass.AP(tensor=base.tensor, offset=base.offset + off,
                ap=[p, [256, 4], [1, 64]])
        for off in (0, 192)
    ]


def _vcols_dr(v_f8, pi, j, ksz):
    """Two destination APs inside v_dr [128, pair, head, parity, 128]:
    even head h -> cols [0,64) of block h*256+j*128, odd -> [64,128)."""
    base = v_f8[:]
    pstr = base.ap[0][0]
    off = base.offset + pi * 2048 + j * 128
    return [
        bass.AP(tensor=base.tensor, offset=off + o2,
                ap=[[pstr, ksz], [512, 4], [1, 64]])
        for o2 in (0, 320)
    ]


def _memset_v_ones(nc, v_f8, v_12, s_v):
    """Write the constant ones columns (= s_v) of the extended V tiles."""
    vb = v_f8[:]
    pstr = vb.ap[0][0]
    # even heads at h*256+64, odd heads at h*256 (+j*128)
    for base_off in (64, 256):
        nc.vector.memset(
            bass.AP(
                tensor=vb.tensor, offset=vb.offset + base_off,
                ap=[[pstr, 128], [2048, len(PAIRS)], [512, 4], [128, 2],
                    [1, 64]],
            ),
            s_v,
        )
    v12b = v_12[:]
    nc.vector.memset(
        bass.AP(
            tensor=v12b.tensor, offset=v12b.offset + 64,
            ap=[[v12b.ap[0][0], 128], [256, 4], [1, 128]],
        ),
        s_v,
    )


def build_kernel(exp_scales, s_v):
    nc = bass.Bass(
        "TRN2", target_bir_lowering=False, debug=False, num_devices=N_CORES
    )

    pat = nc.dram_tensor("pat", [PD, OWN], BF16, kind="ExternalInput").ap()
    addv = nc.dram_tensor("addv", [D, OWN], F32, kind="ExternalInput").ap()
    wembT = nc.dram_tensor("wembT", [PD, D], BF16, kind="ExternalInput").ap()
    wqT = nc.dram_tensor("wqT8", [NL, 128, 2 * DC * 2 * 128], F8,
                         kind="ExternalInput").ap()
    wkT = nc.dram_tensor("wkT8", [NL, 128, 2 * DC * 2 * 128], F8,
                         kind="ExternalInput").ap()
    wvT = nc.dram_tensor("wvxT8", [NL, D, D], F8, kind="ExternalInput").ap()
    woT = nc.dram_tensor("woT", [NL, D, D], BF16, kind="ExternalInput").ap()
    w1T = nc.dram_tensor("w1T", [NL, D, FF], BF16, kind="ExternalInput").ap()
    w2T = nc.dram_tensor("w2T", [NL, FF, D], BF16, kind="ExternalInput").ap()
    bq = nc.dram_tensor("bq", [NL, D], F32, kind="ExternalInput").ap()
    bk = nc.dram_tensor("bk", [NL, D], F32, kind="ExternalInput").ap()
    bv = nc.dram_tensor("bvx", [NL, D], F32, kind="ExternalInput").ap()
    bo = nc.dram_tensor("bo", [NL, D], F32, kind="ExternalInput").ap()
    b1 = nc.dram_tensor("b1", [NL, FF], F32, kind="ExternalInput").ap()
    b2 = nc.dram_tensor("b2", [NL, D], F32, kind="ExternalInput").ap()
    g1 = nc.dram_tensor("g1", [NL, D], F32, kind="ExternalInput").ap()
    be1 = nc.dram_tensor("be1", [NL, D], F32, kind="ExternalInput").ap()
    g2 = nc.dram_tensor("g2", [NL, D], F32, kind="ExternalInput").ap()
    be2 = nc.dram_tensor("be2", [NL, D], F32, kind="ExternalInput").ap()
    ident = nc.dram_tensor("ident", [128, 128], BF16, kind="ExternalInput").ap()
    swapid = nc.dram_tensor("swapid", [128, 128], F32R, kind="ExternalInput").ap()
    xout = nc.dram_tensor("xout", [OWN, D], F32, kind="ExternalOutput").ap()

    with tile.TileContext(nc) as tc:
        from contextlib import ExitStack
        with ExitStack() as _es:
            _pool_spec = [
                ("const", 1, None), ("wsmall", 1, None), ("wff", 2, None),
                ("xp", 2, None), ("xa", 2, None), ("kv", 1, None),
                ("vp", 1, None), ("qo", 1, None), ("zp", 2, None),
                ("zbp", 1, None), ("xs", 1, None), ("big", 1, None),
                ("exps", 6, None), ("stat", 5, None), ("rz", 3, None),
                ("bias", 2, None), ("psA", 1, "PSUM"), ("psB", 2, "PSUM"),
                ("psC", 1, "PSUM"), ("dram", 2, "DRAM"),
            ]
            _pools = {}
            for _nm, _bufs, _space in _pool_spec:
                if _space is None:
                    _pools[_nm] = _es.enter_context(
                        tc.tile_pool(name=_nm, bufs=_bufs)
                    )
                else:
                    _pools[_nm] = _es.enter_context(
                        tc.tile_pool(name=_nm, bufs=_bufs, space=_space)
                    )
            constp = _pools["const"]
            wsmall = _pools["wsmall"]
            wff = _pools["wff"]
            xp = _pools["xp"]
            xap = _pools["xa"]
            kvp = _pools["kv"]
            vp = _pools["vp"]
            qop = _pools["qo"]
            zp = _pools["zp"]
            zbp = _pools["zbp"]
            xsp = _pools["xs"]
            bigp = _pools["big"]
            expp = _pools["exps"]
            statp = _pools["stat"]
            rzp = _pools["rz"]
            biasp = _pools["bias"]
            psA = _pools["psA"]
            psB = _pools["psB"]
            psC = _pools["psC"]
            dramp = _pools["dram"]
            P = dict(
                constp=constp, wsmall=wsmall, wff=wff, xp=xp, xap=xap, kvp=kvp,
                vp=vp, qop=qop, zp=zp, zbp=zbp, xsp=xsp, bigp=bigp, expp=expp,
                statp=statp, rzp=rzp, biasp=biasp, psA=psA, psB=psB, psC=psC,
                dramp=dramp,
            )
            dram_in = dict(
                pat=pat, addv=addv, wembT=wembT, wqT=wqT, wkT=wkT, wvT=wvT,
                woT=woT, w1T=w1T, w2T=w2T, bq=bq, bk=bk, bv=bv, bo=bo, b1=b1,
                swapid=swapid, b2=b2, g1=g1, be1=be1, g2=g2, be2=be2,
                ident=ident, xout=xout,
            )
            ones_bf = constp.tile([128, 128], BF16, name="ones_bf")
            nc.vector.memset(ones_bf[:], 1.0)
            ones_r = constp.tile([128, 128], F32R, name="ones_r")
            nc.vector.tensor_scalar_add(ones_r[:], ones_bf[:], 0.0)
            ident_sb = constp.tile([128, 128], BF16, name="ident_sb")
            nc.sync.dma_start(ident_sb[:], ident[:])
            eps_sb = constp.tile([128, 1], F32, name="eps_sb")
            nc.vector.memset(eps_sb[:], LN_EPS)
            swap_sb = constp.tile([128, 128], F32R, name="swap_sb")
            nc.sync.dma_start(swap_sb[:], swapid[:])
            onesw = constp.tile([128, OWN], F32, name="onesw")
            nc.vector.memset(onesw[:], 1.0)
            P["ones_bf"] = ones_bf
            P["ones_r"] = ones_r
            P["ident_sb"] = ident_sb
            P["eps_sb"] = eps_sb
            P["swap_sb"] = swap_sb
            P["onesw"] = onesw

            # extended-V tiles live for the whole kernel; ones columns are
            # constant (= s_v) and written exactly once here.
            v_f8 = vp.tile([128, len(PAIRS), NH, 2, 128], F8, name="v_f8")
            v_12 = vp.tile([128, NH * 128], F8, name="v_12")
            P["v_f8"] = v_f8
            P["v_12"] = v_12
            _memset_v_ones(nc, v_f8, v_12, s_v)

            x_bf, x_all = _embed(nc, P, dram_in)
            for l in range(NL):
                x_bf, x_all = _one_layer(
                    nc, P, dram_in, l, x_bf, x_all, exp_scales[l]
                )
            _tail(nc, P, dram_in, x_bf)
    return nc


def _embed(nc, P, dr):
    bigp, zp, wff, xp, psB = P["bigp"], P["zp"], P["wff"], P["xp"], P["psB"]
    pat_sb = bigp.tile([128, FTC, OWN], BF16, tag="h", name="pat_sb")
    nc.sync.dma_start(
        pat_sb[:, :PDC, :], dr["pat"].rearrange("(ko p) t -> p ko t", p=128)
    )
    addv_sb = zp.tile([128, DC, OWN], F32, tag="z", name="addv_sb")
    nc.sync.dma_start(addv_sb[:], dr["addv"].rearrange("(co p) t -> p co t", p=128))
    wemb_sb = wff.tile([128, PDC, D], BF16, tag="wff", name="wemb_sb")
    nc.sync.dma_start(wemb_sb[:], dr["wembT"].rearrange("(ko p) d -> p ko d", p=128))

    x_bf = xp.tile([128, DC, OWN], BF16, tag="x", name="x_emb")
    x_all = P["xap"].tile([128, DC, S], F8, tag="xa", name="x_all_0")
    for dt in range(DC):
        for ci in range(2):
            q0 = ci * HQ
            ps = psB.tile([128, 2, 512], F32, tag="s", name="ps_emb")[:, 0, :]
            for kt in range(PDC):
                nc.tensor.matmul(
                    ps[:, :HQ],
                    wemb_sb[:, kt, dt * 128 : (dt + 1) * 128],
                    pat_sb[:, kt, q0 : q0 + HQ],
                    start=(kt == 0),
                    stop=(kt == PDC - 1),
                )
            nc.vector.tensor_tensor(
                x_bf[:, dt, q0 : q0 + HQ], ps[:, :HQ],
                addv_sb[:, dt, q0 : q0 + HQ], OP.add,
            )
            nc.vector.tensor_tensor(
                x_all[:, dt, q0 : q0 + HQ], ps[:, :HQ],
                addv_sb[:, dt, q0 : q0 + HQ], OP.add,
            )
    return x_bf, x_all


def _load_layer_params(nc, P, dr, l):
    biasp, wsmall, wff = P["biasp"], P["wsmall"], P["wff"]
    prm = {}
    for nm in ["bq", "bk", "bo", "b2", "g1", "be1", "g2", "be2"]:
        t = biasp.tile([128, DC], F32, tag=nm, name=nm + "_sb")
        nc.sync.dma_start(t[:], dr[nm][l].rearrange("(o p) -> p o", p=128))
        prm[nm] = t
    b1_sb = biasp.tile([128, FTC], F32, tag="b1", name="b1_sb")
    nc.sync.dma_start(b1_sb[:], dr["b1"][l].rearrange("(o p) -> p o", p=128))
    prm["b1"] = b1_sb
    bv_bc = biasp.tile([128, D], BF16, tag="bvb", name="bv_bc")
    nc.gpsimd.dma_start(bv_bc[:], _bcast_ap(dr["bv"][l]))
    prm["bv_bc"] = bv_bc
    for nm, key in [("wq", "wqT"), ("wk", "wkT")]:
        t = wsmall.tile([128, 2, DC, 2, 128], F8, tag=nm, name=nm + "_sb")
        nc.sync.dma_start(
            t[:].rearrange("p a b c d -> p (a b c d)"), dr[key][l]
        )
        prm[nm] = t
    wo = wsmall.tile([128, DC, D], BF16, tag="wo", name="wo_sb")
    nc.sync.dma_start(wo[:], dr["woT"][l].rearrange("(co p) d -> p co d", p=128))
    prm["wo"] = wo
    wv = wsmall.tile([128, DC, D], F8, tag="wv", name="wv_sb")
    nc.sync.dma_start(wv[:], dr["wvT"][l].rearrange("(co p) d -> p co d", p=128))
    prm["wv"] = wv
    w1_sb = wff.tile([128, DC, FF], BF16, tag="wff", name="w1_sb")
    nc.sync.dma_start(w1_sb[:], dr["w1T"][l].rearrange("(co p) f -> p co f", p=128))
    prm["w1"] = w1_sb
    w2_sb = wff.tile([128, FTC, D], BF16, tag="wff", name="w2_sb")
    nc.sync.dma_start(w2_sb[:], dr["w2T"][l].rearrange("(fo p) d -> p fo d", p=128))
    prm["w2"] = w2_sb
    return prm


def _exchange_start(nc, P, x_bf):
    dramp = P["dramp"]
    xg_in = dramp.tile([2, DC, 128, OWN], BF16, tag="agi", name="xg_in")
    for s_ in range(2):
        nc.sync.dma_start(xg_in[s_].rearrange("c p t -> p c t"), x_bf[:])
    xg_sum = dramp.tile([DC, 128, OWN], BF16, tag="ago", name="xg_sum")
    nc.gpsimd.collective_compute(
        "ReduceScatter",
        OP.add,
        replica_groups=REPLICA_GROUPS,
        ins=[xg_in[:].opt()],
        outs=[xg_sum[:].opt()],
    )
    return xg_sum


def _exchange_finish(nc, P, xg_sum, x_bf, x_all):
    xsum_sb = P["xsp"].tile([128, DC, OWN], BF16, tag="xs", name="xsum_sb")
    nc.sync.dma_start(xsum_sb[:], xg_sum.rearrange("c p t -> p c t"))
    for h in range(2):
        cs = slice(2 * h, 2 * h + 2)
        nc.vector.tensor_tensor(
            x_all[:, cs, OWN:S], xsum_sb[:, cs, :], x_bf[:, cs, :], OP.subtract
        )


def _proj_dr(nc, P, w_f8, x_src, out_sb, bias_sb, t0, tn, act_bias):
    """out_sb[:, dt, t0:t0+tn] (bf16) = fp8 DoubleRow proj + bias."""
    psB = P["psB"]
    hn = tn // 2
    for dt in range(DC):
      for ci in range(2):
            ps = psB.tile([128, 2, 512], F32, tag="s", name="ps_p")[:, 0, :]
            q0 = t0 + ci * hn
            if USE_DR:
                for c2 in range(2):
                    nc.tensor.matmul(
                        ps[:, :hn],
                        w_f8[:, c2, dt, :, :],
                        x_src[:, 2 * c2 : 2 * c2 + 2, q0 : q0 + hn],
                        start=(c2 == 0),
                        stop=(c2 == 1),
                        perf_mode=DR,
                    )
            else:
                for ct in range(DC):
                    nc.tensor.matmul(
                        ps[:, :hn],
                        w_f8[:, ct // 2, dt, ct % 2, :],
                        x_src[:, ct, q0 : q0 + hn],
                        start=(ct == 0),
                        stop=(ct == DC - 1),
                    )
            dst = out_sb[:, dt, q0 : q0 + hn]
            if act_bias:
                nc.scalar.activation(
                    dst, ps[:, :hn], AF.Identity, bias=bias_sb[:, dt : dt + 1]
                )
            else:
                nc.vector.tensor_scalar_add(
                    dst, ps[:, :hn], bias_sb[:, dt : dt + 1]
                )


def _v_proj_tile(nc, P, prm, x_all, t, v_dst_cols, ps_pool_tag):
    """True-V columns for k-tile t (+ bias), fp8 out into v_dr; the ones
    columns are constant (memset once)."""
    k0, ksz = KT[t]
    if ps_pool_tag == "s":
        pst = P["psB"].tile([128, 2, 512], F32, tag="s", name="ps_v")[:, 0, :]
    else:
        pst = P["psA"].tile([128, 512], F32, tag=ps_pool_tag, name="ps_v")
    for ct in range(DC):
        nc.tensor.matmul(
            pst[:ksz, :],
            x_all[:, ct, k0 : k0 + ksz],
            prm["wv"][:, ct, :],
            start=(ct == 0),
            stop=(ct == DC - 1),
        )
    ps_ap = pst[:]
    pstr = ps_ap.ap[0][0]
    bvb = prm["bv_bc"][:]
    for i, (dst, s_off) in enumerate(zip(v_dst_cols, (0, 64))):
        src = bass.AP(tensor=ps_ap.tensor, offset=ps_ap.offset + s_off,
                      ap=[[pstr, ksz], [128, 4], [1, 64]])
        bvs = bass.AP(tensor=bvb.tensor, offset=bvb.offset + s_off,
                      ap=[[bvb.ap[0][0], ksz], [128, 4], [1, 64]])
        nc.vector.tensor_tensor(dst, src, bvs, OP.add)


def _flash_pairs(nc, P, prm, kT, qT, p1, p2, sc_exp, hpair, ci, pis, first_pi):
    psB, expp = P["psB"], P["expp"]
    hdt = hpair
    v_f8 = P["v_f8"]
    q0 = ci * HQ
    for pi in pis:
        ta, tb = PAIRS[pi]
        for sub in range(2):
            hp = sub * 64
            head = 2 * hpair + sub
            e_t = expp.tile([128, 2, HQ], F8, tag="e", name="e_t")
            s_ps = psB.tile([128, 2, 512], F32, tag="s", name="s_ps")
            for j, t in enumerate((ta, tb)):
                k0, ksz = KT[t]
                nc.tensor.matmul(
                    s_ps[:ksz, j, :HQ],
                    kT[hp : hp + 64, hdt, k0 : k0 + ksz],
                    qT[hp : hp + 64, hdt, q0 : q0 + HQ],
                    start=True,
                    stop=True,
                )
            nc.scalar.activation(
                e_t[:, :, :], s_ps[:, :, 0:HQ], AF.Exp, scale=sc_exp
            )
            dst = p1 if sub == 0 else p2
            for j in range(2):
                nc.tensor.matmul(
                    dst[:, :HQ],
                    v_f8[:, pi, head, j, :],
                    e_t[:, j, :],
                    start=(pi == first_pi and j == 0),
                    stop=False,
                )


def _flash_single_and_rz(nc, P, prm, kT, qT, p1, p2, sc_exp, hpair, ci, oT):
    psB, expp, rzp = P["psB"], P["expp"], P["rzp"]
    hdt = hpair
    q0 = ci * HQ
    k0, ksz = KT[SINGLE]
    for sub in range(2):
        hp = sub * 64
        head = 2 * hpair + sub
        e_t = expp.tile([128, 2, HQ], F8, tag="e", name="e_t12")
        s_ps = psB.tile([128, 2, 512], F32, tag="s", name="s_p12")
        nc.tensor.matmul(
            s_ps[:ksz, 0, :HQ],
            kT[hp : hp + 64, hdt, k0 : k0 + ksz],
            qT[hp : hp + 64, hdt, q0 : q0 + HQ],
            start=True,
            stop=True,
        )
        nc.scalar.activation(
            e_t[:ksz, 0, :], s_ps[:ksz, 0, :HQ], AF.Exp, scale=sc_exp
        )
        dst = p1 if sub == 0 else p2
        nc.tensor.matmul(
            dst[:, :HQ],
            P["v_12"][:ksz, head * 128 : (head + 1) * 128],
            e_t[:ksz, 0, :],
            start=False,
            stop=True,
        )
    # 1/Z: Z_o in p2[0:64], Z_e in p1[64:128]; ln+exp base-aligned, then
    # swap halves with the f32r anti-diagonal-identity matmul.
    lnmix = rzp.tile([128, HQ], F32, tag="rz", name="lnmix")
    nc.scalar.activation(lnmix[0:64, :], p2[0:64, :HQ], AF.Ln)
    nc.scalar.activation(lnmix[64:128, :], p1[64:128, :HQ], AF.Ln)
    rzmix = rzp.tile([128, HQ], F32R, tag="rz", name="rzmix")
    nc.scalar.activation(rzmix[:], lnmix[:], AF.Exp, scale=-1.0)
    rsw = psB.tile([128, 2, 512], F32, tag="s", name="rsw")[:, 0, :]
    nc.tensor.matmul(
        rsw[:, :HQ], P["swap_sb"][:], rzmix[:], start=True, stop=True
    )
    rzs = rzp.tile([128, HQ], F32, tag="rz", name="rzs")
    nc.vector.tensor_scalar_add(rzs[:], rsw[:, :HQ], 0.0)
    nc.vector.tensor_tensor(
        oT[0:64, hdt, q0 : q0 + HQ], p1[0:64, :HQ], rzs[0:64, :], OP.mult
    )
    nc.vector.tensor_tensor(
        oT[64:128, hdt, q0 : q0 + HQ], p2[64:128, :HQ], rzs[64:128, :], OP.mult
    )


def _attention_qkv(nc, P, prm, x_all, xg_sum, x_bf):
    """Q/K/V projections with own-first phasing around the exchange."""
    psA, qop = P["psA"], P["qop"]
    qT = qop.tile([128, DC, OWN], BF16, tag="qT", name="qT")
    _proj_dr(nc, P, prm["wq"], x_all, qT, prm["bq"], 0, OWN, act_bias=True)
    kT = P["kvp"].tile([128, DC, S], BF16, tag="kv", name="kT")
    _proj_dr(nc, P, prm["wk"], x_all, kT, prm["bk"], 0, OWN, act_bias=True)
    v_f8 = P["v_f8"]
    for pi in range(3):
        for j in range(2):
            t = PAIRS[pi][j]
            _v_proj_tile(
                nc, P, prm, x_all, t, _vcols_dr(v_f8, pi, j, KT[t][1]),
                "o" if (2 * pi + j) % 2 == 0 else "zz",
            )
    return qT, kT


def _attention_late(nc, P, prm, x_all, xg_sum, x_bf, kT):
    _exchange_finish(nc, P, xg_sum, x_bf, x_all)
    _proj_dr(nc, P, prm["wk"], x_all, kT, prm["bk"], OWN, OWN, act_bias=False)
    v_f8 = P["v_f8"]
    for pi in range(3, len(PAIRS)):
        for j in range(2):
            t = PAIRS[pi][j]
            _v_proj_tile(
                nc, P, prm, x_all, t, _vcols_dr(v_f8, pi, j, KT[t][1]), "s"
            )
    _v_proj_tile(
        nc, P, prm, x_all, SINGLE, _vcols(P["v_12"][:], KT[SINGLE][1]), "s"
    )


def _attention_chunk(nc, P, prm, kT, qT, sc_exp, ci, oT, hpairs):
    psA = P["psA"]
    for hpair in hpairs:
        p1 = psA.tile([128, 512], F32, tag="o", name=f"p1_{ci}_{hpair}")
        p2 = psA.tile([128, 512], F32, tag="zz", name=f"p2_{ci}_{hpair}")
        _flash_pairs(nc, P, prm, kT, qT, p1, p2, sc_exp, hpair, ci,
                     range(len(PAIRS)), 0)
        _flash_single_and_rz(nc, P, prm, kT, qT, p1, p2, sc_exp, hpair, ci, oT)


def _residual_proj(nc, P, w_sb, rhs_T, bias_sb, x_bf, z_out, ci):
    """z_out chunk (f32) = bf16 W^T proj of rhs_T + bias + x_bf residual."""
    psB = P["psB"]
    q0 = ci * HQ
    for dt in range(DC):
        ps = psB.tile([128, 2, 512], F32, tag="s", name="ps_r")[:, 0, :]
        for ct in range(DC):
            nc.tensor.matmul(
                ps[:, :HQ],
                w_sb[:, ct, dt * 128 : (dt + 1) * 128],
                rhs_T[:, ct, q0 : q0 + HQ],
                start=(ct == 0),
                stop=(ct == DC - 1),
            )
        nc.vector.scalar_tensor_tensor(
            z_out[:, dt, q0 : q0 + HQ],
            ps[:, :HQ],
            bias_sb[:, dt : dt + 1],
            x_bf[:, dt, q0 : q0 + HQ],
            OP.add,
            OP.add,
        )


def _ffn(nc, P, prm, x_bf, z_out, ci):
    psC = P["psC"]
    q0 = ci * HQ
    h_bf = P["h_bf"]
    for ft in range(FTC):
        ps = psC.tile(
            [128, 512], F32, tag=("c1" if ft % 2 == 0 else "c2"), name="ps_h"
        )
        for ct in range(DC):
            nc.tensor.matmul(
                ps[:, :HQ],
                prm["w1"][:, ct, ft * 128 : (ft + 1) * 128],
                x_bf[:, ct, q0 : q0 + HQ],
                start=(ct == 0),
                stop=(ct == DC - 1),
            )
        nc.scalar.activation(
            h_bf[:, ft, q0 : q0 + HQ], ps[:, :HQ], AF.Relu,
            bias=prm["b1"][:, ft : ft + 1],
        )
    for dt in range(DC):
        ps2 = psC.tile(
            [128, 512], F32, tag=("c1" if dt % 2 == 0 else "c2"), name="ps_f"
        )
        for ft in range(FTC):
            nc.tensor.matmul(
                ps2[:, :HQ],
                prm["w2"][:, ft, dt * 128 : (dt + 1) * 128],
                h_bf[:, ft, q0 : q0 + HQ],
                start=(ft == 0),
                stop=(ft == FTC - 1),
            )
        nc.vector.scalar_tensor_tensor(
            z_out[:, dt, q0 : q0 + HQ],
            ps2[:, :HQ],
            prm["b2"][:, dt : dt + 1],
            x_bf[:, dt, q0 : q0 + HQ],
            OP.add,
            OP.add,
        )


def _one_layer(nc, P, dr, l, x_bf, x_all, sc_exp):
    xg_sum = _exchange_start(nc, P, x_bf)
    prm = _load_layer_params(nc, P, dr, l)
    qT, kT = _attention_qkv(nc, P, prm, x_all, xg_sum, x_bf)
    oT = P["qop"].tile([128, DC, OWN], BF16, tag="oT", name="oT")

    # chunk A attention; hpair 0 runs own k-tiles while the exchange flies
    psA = P["psA"]
    p1 = psA.tile([128, 512], F32, tag="o", name="p1_a0")
    p2 = psA.tile([128, 512], F32, tag="zz", name="p2_a0")
    _flash_pairs(nc, P, prm, kT, qT, p1, p2, sc_exp, 0, 0, range(0, 3), 0)
    _attention_late(nc, P, prm, x_all, xg_sum, x_bf, kT)
    _flash_pairs(nc, P, prm, kT, qT, p1, p2, sc_exp, 0, 0,
                 range(3, len(PAIRS)), 0)
    _flash_single_and_rz(nc, P, prm, kT, qT, p1, p2, sc_exp, 0, 0, oT)
    _attention_chunk(nc, P, prm, kT, qT, sc_exp, 0, oT, range(1, NH // 2))

    # chunk A tail head (Wo + LN1) before chunk B attention
    z1 = P["zp"].tile([128, DC, OWN], F32R, tag="z", name=f"z1_{l}")
    x_bf1 = P["xp"].tile([128, DC, OWN], BF16, tag="x", name=f"x_ln1_{l}")
    P["h_bf"] = P["bigp"].tile([128, FTC, OWN], BF16, tag="h", name="h_bf")
    _residual_proj(nc, P, prm["wo"], oT, prm["bo"], x_bf, z1, 0)
    _layernorm(nc, P, z1, x_bf1, None, prm["g1"], prm["be1"], 0)

    # chunk B attention overlaps chunk A's FFN
    _attention_chunk(nc, P, prm, kT, qT, sc_exp, 1, oT, range(NH // 2))

    z2 = P["zp"].tile([128, DC, OWN], F32R, tag="z", name=f"z2_{l}")
    last = l == NL - 1
    if last:
        x2 = P["zbp"].tile([128, DC, OWN], F32, tag="xf", name="x_final")
        x_all2 = None
    else:
        x2 = P["xp"].tile([128, DC, OWN], BF16, tag="x", name=f"x_ln2_{l}")
        x_all2 = P["xap"].tile([128, DC, S], F8, tag="xa", name=f"x_all_{l + 1}")
    _ffn(nc, P, prm, x_bf1, z2, 0)
    _layernorm(nc, P, z2, x2, x_all2, prm["g2"], prm["be2"], 0)

    # chunk B tail
    _residual_proj(nc, P, prm["wo"], oT, prm["bo"], x_bf, z1, 1)
    _layernorm(nc, P, z1, x_bf1, None, prm["g1"], prm["be1"], 1)
    _ffn(nc, P, prm, x_bf1, z2, 1)
    _layernorm(nc, P, z2, x2, x_all2, prm["g2"], prm["be2"], 1)
    return x2, x_all2


def _tail(nc, P, dr, x_f32):
    psB = P["psB"]
    xout = dr["xout"]
    ident32 = P["constp"].tile([128, 128], F32, name="ident32")
    nc.vector.tensor_scalar_add(ident32[:], P["ident_sb"][:], 0.0)
    for ti in range(7):
        t0 = ti * 128
        tsz = min(128, OWN - t0)
        xo_sb = P["statp"].tile([128, D], F32, tag="st", name="xo_sb")
        for dt in range(DC):
            tp = psB.tile([128, 2, 512], F32, tag="s", name="tp")[:, 0, :]
            nc.tensor.transpose(
                tp[:tsz, 0:128], x_f32[:, dt, t0 : t0 + tsz], ident32[:]
            )
            nc.vector.tensor_scalar_add(
                xo_sb[:tsz, dt * 128 : (dt + 1) * 128], tp[:tsz, 0:128], 0.0
            )
        nc.sync.dma_start(xout[t0 : t0 + tsz, :], xo_sb[:tsz, :])


def _layernorm(nc, P, z, x_out, x_f8_out, g_sb, be_sb, ci):
    """Post-LN (chunk ci) over features (partition dim), transposed layout."""
    psC, statp, zbp = P["psC"], P["statp"], P["zbp"]
    q0 = ci * HQ
    zf = z[:].bitcast(F32)
    sum_ps = psC.tile([128, 512], F32, tag="c1", name="sum_ps")
    for ct in range(DC):
        nc.tensor.matmul(
            sum_ps[:, :HQ],
            P["ones_r"][:],
            z[:, ct, q0 : q0 + HQ],
            start=(ct == 0),
            stop=(ct == DC - 1),
        )
    sq = zbp.tile([128, DC, HQ], BF16, tag="zb", name="sq_bf")
    for ct in range(DC):
        nc.vector.tensor_tensor(
            sq[:, ct, :], zf[:, ct, q0 : q0 + HQ], zf[:, ct, q0 : q0 + HQ],
            OP.mult,
        )
    sq_ps = psC.tile([128, 512], F32, tag="c2", name="sq_ps")
    for ct in range(DC):
        nc.tensor.matmul(
            sq_ps[:, :HQ],
            P["ones_bf"][:],
            sq[:, ct, :],
            start=(ct == 0),
            stop=(ct == DC - 1),
        )
    mu = statp.tile([128, HQ], F32, tag="st", name="mu")
    nc.vector.tensor_scalar(
        mu[:], sum_ps[:, :HQ], 1.0 / D, None, OP.mult, OP.bypass
    )
    musq = statp.tile([128, HQ], F32, tag="st", name="musq")
    nc.vector.tensor_tensor(musq[:], mu[:], mu[:], OP.mult)
    var = statp.tile([128, HQ], F32, tag="st", name="var")
    nc.vector.scalar_tensor_tensor(
        var[:], sq_ps[:, :HQ], 1.0 / D, musq[:], OP.mult, OP.subtract
    )
    lnv = statp.tile([128, HQ], F32, tag="st", name="lnv")
    nc.scalar.activation(lnv[:], var[:], AF.Ln, bias=P["eps_sb"][:])
    rstd = statp.tile([128, HQ], F32, tag="st", name="rstd")
    nc.scalar.activation(rstd[:], lnv[:], AF.Exp, scale=-0.5)
    mr = statp.tile([128, HQ], F32, tag="st", name="mr")
    nc.vector.tensor_tensor(mr[:], mu[:], rstd[:], OP.mult)
    for ct in range(DC):
        zc = z[:, ct, q0 : q0 + HQ]
        zfc = zf[:, ct, q0 : q0 + HQ]
        nc.vector.tensor_tensor(zc, zfc, rstd[:], OP.mult)
        nc.vector.tensor_tensor(zc, zfc, mr[:], OP.subtract)
        nc.scalar.activation(
            x_out[:, ct, q0 : q0 + HQ],
            zfc,
            AF.Identity,
            bias=be_sb[:, ct : ct + 1],
            scale=g_sb[:, ct : ct + 1],
        )
        if x_f8_out is not None:
            nc.scalar.activation(
                x_f8_out[:, ct, q0 : q0 + HQ],
                zfc,
                AF.Identity,
                bias=be_sb[:, ct : ct + 1],
                scale=g_sb[:, ct : ct + 1],
            )


def _pow2_scale(absmax, target=224.0):
    if absmax <= 0:
        return 1.0
    return 2.0 ** math.floor(math.log2(target / absmax))


def _build_wvx(Wv, s_v):
    """s_v*Wv^T, head-major [NL, D, D] (no zero extension; ones columns of
    the extended V live as constants in SBUF)."""
    f8 = ml_dtypes.float8_e4m3
    return np.ascontiguousarray(Wv.transpose(0, 2, 1) * s_v).astype(f8)


def _build_bvx(bv, s_v):
    return np.asarray(bv, np.float32) * s_v


_NC_CACHE = None
_EXP_SCALES = None
_SV_USED = None


def _host_prep(inputs):
    bf = ml_dtypes.bfloat16
    f8 = ml_dtypes.float8_e4m3
    vid = np.asarray(inputs["vid"], np.float32)
    x = vid.reshape(B, L, C, H // PH, PH, W // PW, PW)
    x = x.transpose(0, 1, 3, 5, 4, 6, 2).reshape(B, L, NP, PD)

    pos = np.asarray(inputs["pos_emb"], np.float32)[0]
    cls = np.asarray(inputs["cls"], np.float32)[0, :, 0, :]
    b_emb = np.asarray(inputs["b_embed"], np.float32)

    Wq = np.asarray(inputs["Wq"], np.float32)
    Wk = np.asarray(inputs["Wk"], np.float32)
    Wv = np.asarray(inputs["Wv"], np.float32)
    s_q = np.array([_pow2_scale(np.abs(Wq[l]).max()) for l in range(NL)])
    s_k = np.array([_pow2_scale(np.abs(Wk[l]).max()) for l in range(NL)])
    s_v = min(SV, min(_pow2_scale(np.abs(Wv[l]).max()) for l in range(NL)))
    global _EXP_SCALES, _SV_USED
    _EXP_SCALES = [float(0.125 / (s_q[l] * s_k[l])) for l in range(NL)]
    _SV_USED = float(s_v)

    def _dr_pack(wT):
        # [NL, c, d] -> [NL, p, c2, dt, parity, col] -> [NL, 128, 2048]
        a = wT.reshape(NL, 2, 2, 128, DC, 128)  # (c2, parity, p, dt, col)
        a = a.transpose(0, 3, 1, 4, 2, 5)  # (p, c2, dt, parity, col)
        return np.ascontiguousarray(a.reshape(NL, 128, 2 * DC * 2 * 128))

    wq8 = _dr_pack(np.ascontiguousarray(Wq.transpose(0, 2, 1))
                   * s_q[:, None, None])
    wk8 = _dr_pack(np.ascontiguousarray(Wk.transpose(0, 2, 1))
                   * s_k[:, None, None])

    shared = {
        "wembT": np.ascontiguousarray(
            np.asarray(inputs["W_embed"], np.float32).T
        ).astype(bf),
        "wqT8": wq8.astype(f8),
        "wkT8": wk8.astype(f8),
        "wvxT8": _build_wvx(Wv, s_v),
        "woT": np.ascontiguousarray(
            np.asarray(inputs["Wo"], np.float32).transpose(0, 2, 1)
        ).astype(bf),
        "w1T": np.ascontiguousarray(
            np.asarray(inputs["W1"], np.float32).transpose(0, 2, 1)
        ).astype(bf),
        "w2T": np.ascontiguousarray(
            np.asarray(inputs["W2"], np.float32).transpose(0, 2, 1)
        ).astype(bf),
        "bq": np.asarray(inputs["bq"], np.float32) * s_q[:, None],
        "bk": np.asarray(inputs["bk"], np.float32) * s_k[:, None],
        "bvx": _build_bvx(np.asarray(inputs["bv"], np.float32), s_v),
        "bo": np.asarray(inputs["bo"], np.float32),
        "b1": np.asarray(inputs["b1"], np.float32),
        "b2": np.asarray(inputs["b2"], np.float32),
        "g1": np.asarray(inputs["ln1_g"], np.float32),
        "be1": np.asarray(inputs["ln1_b"], np.float32),
        "g2": np.asarray(inputs["ln2_g"], np.float32),
        "be2": np.asarray(inputs["ln2_b"], np.float32),
        "ident": np.eye(128, dtype=np.float32).astype(bf),
        "swapid": np.roll(np.eye(128, dtype=np.float32), 64, axis=1),
    }

    in_maps = []
    for c in range(N_CORES):
        b, half = c // 2, c % 2
        f0 = half * (L // 2)
        pat_c = np.zeros((PD, OWN), np.float32)
        addv_c = np.zeros((D, OWN), np.float32)
        for f in range(L // 2):
            fr = f0 + f
            t0 = f * (NP + 1)
            pat_c[:, t0 + 1 : t0 + NP + 1] = x[b, fr].T
            addv_c[:, t0] = pos[fr, 0] + cls[fr]
            addv_c[:, t0 + 1 : t0 + NP + 1] = pos[fr, 1:].T + b_emb[:, None]
        m = {"pat": pat_c.astype(bf), "addv": addv_c}
        m.update(shared)
        in_maps.append(m)
    return in_maps


def kernel(**inputs):
    global _NC_CACHE
    in_maps = _host_prep(inputs)
    if _NC_CACHE is None:
        nc = build_kernel(_EXP_SCALES, _SV_USED)
        legalize_waits(nc)
        _NC_CACHE = nc
    nc = _NC_CACHE
    res = run_bass_kernel_spmd(nc, in_maps, core_ids=list(range(N_CORES)))
    out = np.zeros((B, S, D), np.float32)
    for c in range(N_CORES):
        b, half = c // 2, c % 2
        out[b, half * OWN : (half + 1) * OWN, :] = res.results[c]["xout"]
    return out
